# revision 33
# baseline (speedup 1.0000x reference)
#!/usr/bin/env python3
"""Multi-head attention (B=16, N=1024, E=768, H=8, softmax-then-scale variant)
as a Bass/Tile kernel on 8 TRN2 NeuronCores, data-parallel over the batch.

Per core (2 batch elements, T=2048 tokens):
  - QK projections and the V build run as fp8(e4m3) DoubleRow matmuls with a
    2-term residual split of both operands (a ~= a8 + da8 at a common scale;
    3 of the 4 cross terms are kept, the lo*lo term ~0.1% is dropped). The
    host supplies x8/dx8 at scale 8 and W8/dWlo at scale 32; the 1/256
    descale is folded into the PSUM->SBUF copies. DoubleRow contracts
    2x128 K per instruction at 0.5 cycles/row, so each term runs at 4x the
    fp32r rate and the 3-term total is 0.75x.
  - energy / attn@V / output projection stay fp32r (full-rate PE): exp
    values span e^54 so fp8 can't represent the attention weights, and the
    softmax amplifies any q/k quantization into argmax flips.
  - loop over batch b, then head h:
      energy^T per ktile: lhsT = K^T slice [96,128], rhs = Q^T [96,512]
      exp on ScalarE (no max subtraction: |energy| <~ 60 fits fp32 exp)
      attn@V flash-style: lhsT = Vhat [128, 97] (V cols for head h + a
        sqrt(E) constant column so row 96 accumulates sqrt(E)*sumexp),
        rhs = expT [128,512], accumulated over 8 k-tiles -> zT [97, 1024]
      normalize: recip = 1/zT[96] (DVE), replicated across partitions by
        the gpsimd partition_broadcast custom op, z_h = zT[0:96] * recip
    then output projection for batch b: R = sum_h z_h^T.T @ Wo_h + 1^T bo
"""
import os
import sys

sys.path.insert(0, "/opt/trn_rl_repo")

import numpy as np

B, N, E, H, D = 16, 1024, 768, 8, 96
NCORES = 8
BPC = B // NCORES          # batch elements per core
T = BPC * N                # tokens per core
KT2 = E // 256             # DoubleRow k-tiles over embedding dim (3)
MT = T // 128              # token tiles per core (16)
NKT = N // 128             # k-tiles over sequence (8)
SXW = 256.0                # x scale (8) * W scale (32)

_CACHE = {}


def _build(with_bias=True):
    import concourse.tile as tile
    from concourse import bacc, mybir

    f32 = mybir.dt.float32
    f32r = mybir.dt.float32r
    f8 = mybir.dt.float8e4

    nc = bacc.Bacc("TRN2", target_bir_lowering=False, debug=False)

    # fp8 operand pairs (hi, lo) for x, Wq/Wk, Wv; f32r elsewhere. Layouts
    # are pre-packed on the host for DoubleRow ([partition, 2, free] slices).
    x8_d = nc.dram_tensor("x8", [2, 128, KT2, 2, T], f8, kind="ExternalInput").ap()
    wqk_d = nc.dram_tensor("wqk8", [2, 128, H, 2, KT2, 2, D], f8,
                           kind="ExternalInput").ap()
    wv_d = nc.dram_tensor("wv8", [2, 128, KT2, 2, E], f8,
                          kind="ExternalInput").ap()
    wo_d = nc.dram_tensor("wo8", [2, D, H // 2, 2, E], f8,
                          kind="ExternalInput").ap()
    bqk_d = nc.dram_tensor("bqk", [D, 2 * H], f32, kind="ExternalInput").ap()
    bv_d = nc.dram_tensor("bv1", [1, E], f32r, kind="ExternalInput").ap()
    bo_d = nc.dram_tensor("bo1", [1, E], f32r, kind="ExternalInput").ap()
    out_d = nc.dram_tensor("out", [T, E], f32, kind="ExternalOutput").ap()

    with tile.TileContext(nc) as tc:
        _body(nc, tc, mybir,
              x8_d, wqk_d, wv_d, wo_d, bqk_d, bv_d, bo_d, out_d,
              with_bias)

    nc.compile()
    return nc


def _body(nc, tc, mybir,
          x8_d, wqk_d, wv_d, wo_d, bqk_d, bv_d, bo_d, out_d,
          with_bias):
    from contextlib import ExitStack
    from concourse import library_config

    f32 = mybir.dt.float32
    f32r = mybir.dt.float32r
    f8 = mybir.dt.float8e4
    Exp = mybir.ActivationFunctionType.Exp
    ADD = mybir.AluOpType.add
    MULT = mybir.AluOpType.mult
    DR = mybir.MatmulPerfMode.DoubleRow
    SUB = mybir.AluOpType.subtract
    # the Vhat constant column is sqrt(E)/SZ so the normalize reciprocal
    # yields SZ/(sqrt(E)*sumexp): z lands pre-scaled by SZ for fp8 storage
    SZ = 256.0
    SQRT_E = float(np.float32(np.sqrt(E))) / SZ
    INV = 1.0 / SXW
    INVO = 1.0 / (SZ * 32.0)   # descale for the output projection copy

    ctx = ExitStack()
    with ctx:
        persist = ctx.enter_context(tc.tile_pool(name="persist", bufs=1))
        qkpool = ctx.enter_context(tc.tile_pool(name="qkpool", bufs=1))
        projp = ctx.enter_context(tc.tile_pool(name="projp", bufs=2, space="PSUM"))
        epp = ctx.enter_context(tc.tile_pool(name="epp", bufs=2, space="PSUM"))
        zp = ctx.enter_context(tc.tile_pool(name="zp", bufs=2, space="PSUM"))

        xt = []                 # [(hi tiles), (lo tiles)] per KT2
        vhat = []
        wo8 = []
        state = {}

        # ---------------- helpers ----------------
        def proj_head(b, h):
            """Q^T/K^T for (b, h): 9 DoubleRow fp8 matmuls per 512-col chunk
            (terms W8*x8 + W8*dx8 + dWlo*x8), then a descaling copy."""
            tok0 = b * N
            qk = {}
            for nm in ("q", "k"):
                qk[nm] = qkpool.tile([D, N], f32r, name=f"{nm}t", tag=f"{nm}t",
                                     bufs=2)
            for wi, nm in enumerate(("q", "k")):
                wtile = state[f"w{nm}8"]
                qt = qk[nm]
                for tc2 in range(N // 512):
                    pq = projp.tile([128, 512], f32, name="pp", tag="pp")
                    sl = slice(tok0 + tc2 * 512, tok0 + (tc2 + 1) * 512)
                    first = True
                    for wt, xterm in ((0, 0), (0, 1), (1, 0)):
                        for c in range(KT2):
                            nc.tensor.matmul(
                                pq[0:D, :],
                                wtile[:, h][:, wt][:, c],
                                xt[xterm][c][:, :, sl],
                                start=first,
                                stop=(wt == 1 and c == KT2 - 1),
                                perf_mode=DR,
                            )
                            first = False
                    if with_bias:
                        nc.vector.tensor_scalar(
                            out=qt[:, tc2 * 512:(tc2 + 1) * 512],
                            in0=pq[0:D, :],
                            scalar1=INV,
                            scalar2=state["bqk_t"][:, wi * H + h:wi * H + h + 1],
                            op0=MULT, op1=ADD,
                        )
                    else:
                        nc.vector.tensor_scalar(
                            out=qt[:, tc2 * 512:(tc2 + 1) * 512],
                            in0=pq[0:D, :],
                            scalar1=INV, scalar2=None, op0=MULT,
                        )
            return qk

        def attention(b, h, qk, zpairs, narrow=False):
            """energy -> exp -> attn@V -> normalized fp8 z split for (b, h)."""
            zT = zp.tile([128, N], f32, name="zT", tag="zT")
            for kt in range(NKT):
                ext = expp.tile([128, N], f32r, name="ext", tag="ext")
                for qc in range(2):
                    ep = epp.tile([128, 512], f32, name="ep", tag="ep")
                    nc.tensor.matmul(
                        ep,
                        qk["k"][:, kt * 128:(kt + 1) * 128],
                        qk["q"][:, qc * 512:(qc + 1) * 512],
                        start=True, stop=True,
                    )
                    nc.scalar.activation(
                        out=ext[:, qc * 512:(qc + 1) * 512], in_=ep, func=Exp)
                    nc.tensor.matmul(
                        zT[0:D + 1, qc * 512:(qc + 1) * 512],
                        vhat[b * NKT + kt][:, h, :],
                        ext[:, qc * 512:(qc + 1) * 512],
                        start=(kt == 0), stop=(kt == NKT - 1),
                    )

            # normalize: z = SZ * zT[0:D] / (sqrt(E)*sumexp), then split into
            # fp8 hi/lo for the DoubleRow output projection. Wide
            # reciprocal+broadcast for early heads (throughput); narrow
            # per-chunk chains for the close pair (latency — the output
            # projection close matmuls wait on these).
            zhi, zlo = zpairs
            pj, slot = h // 2, h % 2
            nchunk = 2 if narrow else 1
            w = N // nchunk
            sfx = "n" if narrow else "w"
            for ch in range(nchunk):
                csl = slice(ch * w, (ch + 1) * w)
                recip = rbp.tile([1, w], f32, name="recip", tag=f"recip{sfx}",
                                 bufs=2)
                nc.vector.reciprocal(out=recip, in_=zT[D:D + 1, csl])
                rb = rbp.tile([D, w], f32, name="rb", tag=f"rb{sfx}",
                              bufs=2)
                nc.gpsimd.partition_broadcast(out_ap=rb, in_ap=recip)
                for qc in range(w // 512):
                    sl = slice(ch * w + qc * 512, ch * w + (qc + 1) * 512)
                    t = rbp.tile([D, 512], f32, name="zt_t", tag="zt_t")
                    nc.vector.tensor_mul(
                        out=t, in0=zT[0:D, sl], in1=rb[:, qc * 512:(qc + 1) * 512])
                    nc.vector.tensor_copy(out=zhi[pj][:, slot, sl], in_=t)
                    nc.vector.tensor_tensor(
                        out=zlo[pj][:, slot, sl], in0=t,
                        in1=zhi[pj][:, slot, sl], op=SUB)

        def final_proj(b, zpairs, jlast):
            """fp8 DoubleRow output projection over head pairs, software-
            pipelined: the early-ready pairs of several groups are accumulated
            before the first jlast-pair matmul so the PE has work while the
            last heads' normalize chains still run."""
            zhi, zlo = zpairs
            tok0 = b * N
            groups = [(mt, half) for mt in range(NKT) for half in range(2)]
            DEPTH = 6 if b == BPC - 1 else 5
            NP = H // 2
            jopen = [j for j in range(NP) if j != jlast]
            prs = {}
            ros = {}

            def open_group(g):
                mt, half = groups[g]
                k = g % DEPTH
                if k < 2:
                    pr = projp.tile([128, 384], f32, name="pp", tag="pp")
                elif k < 4:
                    pr = epp.tile([128, 384], f32, name="fep", tag="ep")
                else:
                    pr = zp.tile([128, 384], f32, name="fzt", tag="zT")
                cols = slice(half * 384, (half + 1) * 384)
                msl = slice(mt * 128, (mt + 1) * 128)
                first = True
                for j in jopen:
                    for zt, wt in ((zhi[j], 0), (zlo[j], 0), (zhi[j], 1)):
                        nc.tensor.matmul(
                            pr, zt[:, :, msl], wo8[wt][:, j][:, :, cols],
                            start=first, stop=False, perf_mode=DR,
                        )
                        first = False
                prs[g] = pr

            for g in range(min(DEPTH, len(groups))):
                open_group(g)
            for g, (mt, half) in enumerate(groups):
                pr = prs.pop(g)
                cols = slice(half * 384, (half + 1) * 384)
                msl = slice(mt * 128, (mt + 1) * 128)
                j = jlast
                for ti, (zt, wt) in enumerate(
                        ((zhi[j], 0), (zlo[j], 0), (zhi[j], 1))):
                    nc.tensor.matmul(
                        pr, zt[:, :, msl], wo8[wt][:, j][:, :, cols],
                        start=False,
                        stop=(ti == 2 and not with_bias), perf_mode=DR,
                    )
                if with_bias:
                    # bo is pre-scaled by SZ*32 on the host
                    nc.tensor.matmul(
                        pr, onescol_r, state["bor"][:, cols],
                        start=False, stop=True,
                    )
                if half == 0:
                    ros[mt] = rop.tile([128, E], f32, name="ro", tag="ro")
                if g % 2 == 0:
                    nc.scalar.mul(out=ros[mt][:, cols], in_=pr, mul=INVO)
                else:
                    nc.vector.tensor_scalar(
                        out=ros[mt][:, cols], in0=pr,
                        scalar1=INVO, scalar2=None, op0=MULT)
                if g + DEPTH < len(groups):
                    open_group(g + DEPTH)
                # alternate the output stores across the two HWDGE queues so
                # the end-of-kernel DMA tail is not serialized on one queue
                dma_eng = nc.sync if g % 2 == 0 else nc.scalar
                dma_eng.dma_start(
                    out=out_d[tok0 + mt * 128:tok0 + (mt + 1) * 128, cols],
                    in_=ros[mt][:, cols])
                if half == 1:
                    ros.pop(mt)

        # ---------------- phase 0: loads + Vhat ----------------
        qk00 = None
        with tc.tile_pool(name="wvpool", bufs=1) as wvpool:
            for term in range(2):
                tiles = []
                for c in range(KT2):
                    tiles.append(persist.tile([128, 2, T], f8,
                                              name=f"xt{term}_{c}",
                                              tag=f"xt{term}_{c}"))
                xt.append(tiles)

            def load_x(sl):
                for term in range(2):
                    for c in range(KT2):
                        nc.sync.dma_start(
                            out=xt[term][c][:, :, sl],
                            in_=x8_d[term][:, c][:, :, sl])

            # constants
            ones_f = persist.tile([1, 128], f32, name="ones_f", tag="ones_f")
            nc.vector.memset(ones_f, 1.0)
            onescol_r = persist.tile([1, 128], f32r, name="ones_r", tag="ones_r")
            nc.vector.tensor_copy(out=onescol_r, in_=ones_f)
            c27f = persist.tile([128, 1], f32, name="c27f", tag="c27f")
            nc.vector.memset(c27f, SQRT_E)
            c27r = persist.tile([128, 1], f32r, name="c27r", tag="c27r")
            nc.vector.tensor_copy(out=c27r, in_=c27f)

            # first x quarter interleaved with Wv so the Vhat(0) psum
            # group can start accumulating early; hi terms are loaded before
            # lo terms to match the matmul emission order within each group
            # Wv/Wq/Wk on the gpsimd queue (the scalar HWDGE queue would
            # head-of-line block the Vhat copies on the ACT sequencer).
            # Wv column halves first so the first Vhat group unblocks early.
            wv = []
            for term in range(2):
                wv.append(wvpool.tile([128, KT2, 2, E], f8, name=f"wv{term}",
                                      tag=f"wv{term}"))
            for term in range(2):
                nc.gpsimd.dma_start(out=wv[term][:, :, :, 0:4 * D],
                                    in_=wv_d[term][:, :, :, 0:4 * D])
            load_x(slice(0, 256))
            load_x(slice(256, 512))
            for nm, wi in (("q", 0), ("k", 1)):
                state[f"w{nm}8"] = persist.tile(
                    [128, H, 2, KT2, 2, D], f8, name=f"w{nm}8", tag=f"w{nm}8")
            for h in (6, 7, 0, 1, 2, 3, 4, 5):
                for nm, wi in (("q", 0), ("k", 1)):
                    nc.gpsimd.dma_start(out=state[f"w{nm}8"][:, h],
                                        in_=wqk_d[wi][:, h])
            for term in range(2):
                nc.gpsimd.dma_start(out=wv[term][:, :, :, 4 * D:E],
                                    in_=wv_d[term][:, :, :, 4 * D:E])
            load_x(slice(512, 1024))

            # gpsimd ucode library with partition_broadcast (needed by the
            # first normalize)
            nc.gpsimd.load_library(library_config.attn)
            load_x(slice(1024, 1536))
            load_x(slice(1536, 2048))

            # biases
            bqk_t = persist.tile([D, 2 * H], f32, name="bqk_t", tag="bqk_t")
            nc.gpsimd.dma_start(out=bqk_t, in_=bqk_d)
            state["bqk_t"] = bqk_t
            bvr = persist.tile([1, E], f32r, name="bvr", tag="bvr")
            nc.gpsimd.dma_start(out=bvr, in_=bv_d)

            def build_vhat(mt):
                # Vhat[mt] : [128 tokens, H, D+1]; column D holds sqrt(E)
                vh = persist.tile([128, H, D + 1], f32r, name=f"vhat{mt}",
                                  tag=f"vhat{mt}")
                msl = slice(mt * 128, (mt + 1) * 128)
                for half in range(2):  # heads 0-3 / 4-7 (384 cols each)
                    pv = projp.tile([128, 512], f32, name="pp", tag="pp")
                    cols = slice(half * 4 * D, (half + 1) * 4 * D)
                    first = True
                    # dx8 last: its DMA queue (scalar) lags the x8 loads
                    for wt, xterm in ((0, 0), (1, 0), (0, 1)):
                        for c in range(KT2):
                            nc.tensor.matmul(
                                pv[:, 0:4 * D],
                                xt[xterm][c][:, :, msl],
                                wv[wt][:, c][:, :, cols],
                                start=first,
                                stop=(with_bias is False and wt == 1
                                      and c == KT2 - 1),
                                perf_mode=DR,
                            )
                            first = False
                    if with_bias:
                        # bv is pre-scaled by SXW on the host
                        nc.tensor.matmul(
                            pv[:, 0:4 * D], onescol_r, bvr[:, cols],
                            start=False, stop=True,
                        )
                    if mt < 8:
                        nc.scalar.mul(
                            out=vh[:, half * 4:(half + 1) * 4, 0:D],
                            in_=pv[:, 0:4 * D].rearrange("p (h d) -> p h d",
                                                         h=4),
                            mul=INV,
                        )
                    else:
                        nc.vector.tensor_scalar(
                            out=vh[:, half * 4:(half + 1) * 4, 0:D],
                            in0=pv[:, 0:4 * D].rearrange("p (h d) -> p h d",
                                                         h=4),
                            scalar1=INV, scalar2=None, op0=MULT,
                        )
                nc.vector.tensor_copy(
                    out=vh[:, :, D:D + 1],
                    in_=c27r.to_broadcast([128, H, 1]),
                )
                vhat.append(vh)

            # Vhat 0-7 (x half 0), then the first head projection (keeps the
            # PE busy while half 1 streams in), then Vhat 8-15
            for mt in range(8):
                build_vhat(mt)
            qk00 = proj_head(0, 6)
            for mt in range(8, 16):
                build_vhat(mt)

        # stage + wv pools released; later pools reuse their space
        expp = ctx.enter_context(tc.tile_pool(name="expp", bufs=3))
        rbp = ctx.enter_context(tc.tile_pool(name="rbp", bufs=2))
        rop = ctx.enter_context(tc.tile_pool(name="rop", bufs=2))
        ztpool = ctx.enter_context(tc.tile_pool(name="ztpool", bufs=1))
        wopool = ctx.enter_context(tc.tile_pool(name="wopool", bufs=1))

        # Wo -> fp8 hi/lo per-head-pair tiles + bo (phase 2 operands)
        for term in range(2):
            wot = wopool.tile([D, H // 2, 2, E], f8, name=f"wo{term}",
                              tag=f"wo{term}")
            nc.gpsimd.dma_start(out=wot, in_=wo_d[term])
            wo8.append(wot)
        if with_bias:
            bor = wopool.tile([1, E], f32r, name="bor", tag="bor")
            nc.gpsimd.dma_start(out=bor, in_=bo_d)
            state["bor"] = bor

        # ---------------- phases 1+2, batch-major, software-pipelined ------
        # head order: pair 3 (h6,h7) first so it is ready long before the
        # output projection; pair 2 (h4,h5) finishes last and is the close
        # pair there. Each head's projection is emitted one step ahead so
        # the energy matmuls never wait on the PSUM->SBUF copy latency.
        ORDER = (6, 7, 0, 1, 2, 3, 4, 5)
        qk_next = {ORDER[0]: qk00}
        for b in range(BPC):
            zhi, zlo = [], []
            for j in range(H // 2):
                zhi.append(ztpool.tile([D, 2, N], f8, name=f"z8_{j}",
                                       tag=f"z8_{j}", bufs=2))
                zlo.append(ztpool.tile([D, 2, N], f8, name=f"dz8_{j}",
                                       tag=f"dz8_{j}", bufs=2))
            zpairs = (zhi, zlo)
            for idx, h in enumerate(ORDER):
                qk = qk_next.pop(h) if h in qk_next else proj_head(b, h)
                attention(b, h, qk, zpairs, narrow=(h // 2 == 2))
            qk_next = {}
            if b + 1 < BPC:
                # emit next batch's first projection before the output
                # projection so the PE has work while the last z normalizes
                qk_next[ORDER[0]] = proj_head(b + 1, ORDER[0])
            final_proj(b, zpairs, jlast=2)


def _get_runner(with_bias=False):
    """Build (once per variant) a jitted shard_map executing the NEFF."""
    key = ("runner", with_bias)
    if key in _CACHE:
        return _CACHE[key]

    import jax
    from jax.experimental.shard_map import shard_map
    from jax.sharding import Mesh, NamedSharding, PartitionSpec
    from concourse import mybir
    from concourse.bass2jax import (
        _bass_exec_p, install_neuronx_cc_hook, partition_id_tensor)

    nc = _build(with_bias=with_bias)
    install_neuronx_cc_hook()

    partition_name = (
        nc.partition_id_tensor.name if nc.partition_id_tensor else None)
    in_names, out_names, out_avals, zero_outs = [], [], [], []
    for alloc in nc.m.functions[0].allocations:
        if not isinstance(alloc, mybir.MemoryLocationSet):
            continue
        name = alloc.memorylocations[0].name
        if alloc.kind == "ExternalInput":
            if name != partition_name:
                in_names.append(name)
        elif alloc.kind == "ExternalOutput":
            out_names.append(name)
            shape = tuple(alloc.tensor_shape)
            dtype = mybir.dt.np(alloc.dtype)
            out_avals.append(jax.core.ShapedArray(shape, dtype))
            zero_outs.append(np.zeros(shape, dtype))
    n_params = len(in_names)
    all_in_names = in_names + out_names
    if partition_name is not None:
        all_in_names = all_in_names + [partition_name]

    def _bass_body(*args):
        operands = list(args)
        if partition_name is not None:
            operands.append(partition_id_tensor())
        outs = _bass_exec_p.bind(
            *operands,
            out_avals=tuple(out_avals),
            in_names=tuple(all_in_names),
            out_names=tuple(out_names),
            lowering_input_output_aliases=(),
            sim_require_finite=True,
            sim_require_nnan=True,
            nc=nc,
        )
        return tuple(outs)

    devices = jax.devices()[:NCORES]
    mesh = Mesh(np.asarray(devices), ("core",))
    spec = PartitionSpec("core")
    rspec = PartitionSpec()          # replicated (weights/biases)
    sharding = NamedSharding(mesh, spec)
    rsharding = NamedSharding(mesh, rspec)
    n_outs = len(out_names)
    # x8 is per-core data; everything else is identical across cores
    in_specs = tuple(spec if nm == "x8" else rspec for nm in in_names)
    jitted = jax.jit(
        shard_map(
            _bass_body, mesh=mesh,
            in_specs=in_specs + (spec,) * n_outs,
            out_specs=(spec,) * n_outs,
            check_rep=False,
        ),
        keep_unused=True,
    )
    zeros_dev = [
        jax.device_put(np.concatenate([z] * NCORES, axis=0), sharding)
        for z in zero_outs
    ]
    runner = {
        "jitted": jitted, "in_names": in_names, "out_names": out_names,
        "sharding": sharding, "rsharding": rsharding,
        "zeros_dev": zeros_dev, "jax": jax,
    }
    _CACHE[key] = runner
    return runner


def _split8(a, s):
    """fp8 residual split: return (hi, lo) e4m3 arrays with hi+lo ~= s*a."""
    import ml_dtypes
    E4 = ml_dtypes.float8_e4m3
    sa = (s * a).astype(np.float32)
    hi = sa.astype(E4)
    lo = (sa - hi.astype(np.float32)).astype(E4)
    return hi, lo


def _prep_inputs(x, Wq, bq, Wk, bk, Wv, bv, Wo, bo):
    """Host-side prep: arrays keyed by NEFF input name."""
    import ml_dtypes
    x = np.asarray(x, dtype=np.float32)
    Wq, Wk, Wv, Wo = (np.asarray(w, dtype=np.float32) for w in (Wq, Wk, Wv, Wo))
    bq, bk, bv, bo = (np.asarray(v, dtype=np.float32) for v in (bq, bk, bv, bo))

    # x: [B,N,E] -> per-core xT [E,T] -> x8[term, p, c, i, t] fp8 at scale 8
    xT = x.reshape(NCORES, T, E).transpose(0, 2, 1)     # [NC, E, T]
    xh, xl = _split8(xT, 8.0)
    # [NC, E, T] -> [NC, c, 2, 128, T] -> [NC, 128, c, 2, T]
    def xlay(a):
        return np.ascontiguousarray(
            a.reshape(NCORES, KT2, 2, 128, T).transpose(0, 3, 1, 2, 4))
    # concat over cores on the leading (sharded) axis
    x8 = np.stack([xlay(xh), xlay(xl)], axis=1).reshape(
        NCORES * 2, 128, KT2, 2, T)

    # Wq/Wk: [E, E] -> [2(qk), 128, H, 2(term), KT2, 2, D] fp8 at scale 32
    def wlay(W):
        hi, lo = _split8(W, 32.0)
        # rows k = c*256 + i*128 + p
        def lay(a):
            return a.reshape(KT2, 2, 128, H, D).transpose(2, 3, 0, 1, 4)
        # -> [128, H, KT2, 2, D]; stack terms -> [128, H, 2, KT2, 2, D]
        return np.stack([lay(hi), lay(lo)], axis=2)
    wqk = np.ascontiguousarray(np.stack([wlay(Wq), wlay(Wk)], axis=0))

    # Wv: [E, E] -> [2(term), 128, KT2, 2, E] at scale 32
    vh, vl = _split8(Wv, 32.0)
    def vlay(a):
        return a.reshape(KT2, 2, 128, E).transpose(2, 0, 1, 3)
    wv8 = np.ascontiguousarray(np.stack([vlay(vh), vlay(vl)], axis=0))

    # Wo: [E, E] -> [2(term), D, H/2(pair), 2, E] at scale 32; slot i of
    # pair j holds rows for head 2j+i
    oh, ol = _split8(Wo, 32.0)
    def olay(a):
        return a.reshape(H // 2, 2, D, E).transpose(2, 0, 1, 3)
    wo8 = np.ascontiguousarray(np.stack([olay(oh), olay(ol)], axis=0))

    bqk = np.ascontiguousarray(
        np.concatenate([bq.reshape(H, D).T, bk.reshape(H, D).T], axis=1))

    return {
        "x8": x8, "wqk8": wqk, "wv8": wv8, "wo8": wo8,
        "bqk": bqk,
        "bv1": np.ascontiguousarray(SXW * bv.reshape(1, E)),
        "bo1": np.ascontiguousarray(256.0 * 32.0 * bo.reshape(1, E)),
    }


def _run(inputs, device_resident=None, with_bias=False):
    r = _get_runner(with_bias)
    args = []
    for name in r["in_names"]:
        if device_resident is not None and name in device_resident:
            args.append(device_resident[name])
        else:
            args.append(inputs[name])
    outs = r["jitted"](*args, *r["zeros_dev"])
    return {name: outs[i] for i, name in enumerate(r["out_names"])}


def _weights_on_device(inputs, with_bias=False):
    """device_put the (replicated) weight/bias arrays once per unique value."""
    import hashlib
    r = _get_runner(with_bias)
    key = hashlib.sha1()
    for name in sorted(inputs):
        if name == "x8":
            continue
        a = inputs[name]
        key.update(name.encode())
        key.update(a.shape.__repr__().encode())
        key.update(a.tobytes())
    key = key.hexdigest()
    cached = _CACHE.get("weights_dev")
    if cached is not None and cached[0] == key:
        return cached[1]
    dev = {
        name: r["jax"].device_put(a, r["rsharding"])
        for name, a in inputs.items() if name != "x8"
    }
    _CACHE["weights_dev"] = (key, dev)
    return dev


def kernel(x, Wq, bq, Wk, bk, Wv, bv, Wo, bo):
    with_bias = any(
        np.any(np.asarray(v)) for v in (bq, bk, bv, bo))
    inputs = _prep_inputs(x, Wq, bq, Wk, bk, Wv, bv, Wo, bo)
    dev = _weights_on_device(inputs, with_bias)
    outs = _run(inputs, dev, with_bias)
    out = np.asarray(outs["out"])          # [NCORES*T, E]
    return out.reshape(B, N, E)


def bench(x, Wq, bq, Wk, bk, Wv, bv, Wo, bo, iters=20):
    """Time repeated executions with all inputs device-resident."""
    import time
    r = _get_runner()
    inputs = _prep_inputs(x, Wq, bq, Wk, bk, Wv, bv, Wo, bo)
    dev = _weights_on_device(inputs)
    dev = dict(dev)
    dev["x8"] = r["jax"].device_put(inputs["x8"], r["sharding"])

    out = _run(inputs, dev)
    list(out.values())[0].block_until_ready()

    t0 = time.time()
    last = None
    for _ in range(iters):
        last = _run(inputs, dev)
    for v in last.values():
        v.block_until_ready()
    dt = (time.time() - t0) / iters
    return dt


# revision 34
# speedup vs baseline: 1.0666x; 1.0666x over previous
#!/usr/bin/env python3
"""Multi-head attention (B=16, N=1024, E=768, H=8, softmax-then-scale variant)
as a Bass/Tile kernel on 8 TRN2 NeuronCores, data-parallel over the batch.

Per core (2 batch elements, T=2048 tokens):
  - QK projections and the V build run as fp8(e4m3) DoubleRow matmuls with a
    2-term residual split of both operands (a ~= a8 + da8 at a common scale;
    3 of the 4 cross terms are kept, the lo*lo term ~0.1% is dropped). The
    host supplies x8/dx8 at scale 8 and W8/dWlo at scale 32; the 1/256
    descale is folded into the PSUM->SBUF copies. DoubleRow contracts
    2x128 K per instruction at 0.5 cycles/row, so each term runs at 4x the
    fp32r rate and the 3-term total is 0.75x.
  - energy / attn@V / output projection stay fp32r (full-rate PE): exp
    values span e^54 so fp8 can't represent the attention weights, and the
    softmax amplifies any q/k quantization into argmax flips.
  - loop over batch b, then head h:
      energy^T per ktile: lhsT = K^T slice [96,128], rhs = Q^T [96,512]
      exp on ScalarE (no max subtraction: |energy| <~ 60 fits fp32 exp)
      attn@V flash-style: lhsT = Vhat [128, 97] (V cols for head h + a
        sqrt(E) constant column so row 96 accumulates sqrt(E)*sumexp),
        rhs = expT [128,512], accumulated over 8 k-tiles -> zT [97, 1024]
      normalize: recip = 1/zT[96] (DVE), replicated across partitions by
        the gpsimd partition_broadcast custom op, z_h = zT[0:96] * recip
    then output projection for batch b: R = sum_h z_h^T.T @ Wo_h + 1^T bo
"""
import os
import sys

sys.path.insert(0, "/opt/trn_rl_repo")

import numpy as np

B, N, E, H, D = 16, 1024, 768, 8, 96
NCORES = 8
BPC = B // NCORES          # batch elements per core
T = BPC * N                # tokens per core
KT2 = E // 256             # DoubleRow k-tiles over embedding dim (3)
MT = T // 128              # token tiles per core (16)
NKT = N // 128             # k-tiles over sequence (8)
SXW = 256.0                # x scale (8) * W scale (32)

_CACHE = {}


def _build(with_bias=True):
    import concourse.tile as tile
    from concourse import bacc, mybir

    f32 = mybir.dt.float32
    f32r = mybir.dt.float32r
    f8 = mybir.dt.float8e4

    nc = bacc.Bacc("TRN2", target_bir_lowering=False, debug=False)

    # fp8 operand pairs (hi, lo) for x, Wq/Wk, Wv; f32r elsewhere. Layouts
    # are pre-packed on the host for DoubleRow ([partition, 2, free] slices).
    x8_d = nc.dram_tensor("x8", [2, 128, KT2, 2, T], f8, kind="ExternalInput").ap()
    wqk_d = nc.dram_tensor("wqk8", [2, 128, H, 2, KT2, 2, D], f8,
                           kind="ExternalInput").ap()
    wv_d = nc.dram_tensor("wv8", [2, 128, KT2, 2, E], f8,
                          kind="ExternalInput").ap()
    wo_d = nc.dram_tensor("wo8", [2, D, H // 2, 2, E], f8,
                          kind="ExternalInput").ap()
    bqk_d = nc.dram_tensor("bqk", [D, 2 * H], f32, kind="ExternalInput").ap()
    bv_d = nc.dram_tensor("bv1", [1, E], f32r, kind="ExternalInput").ap()
    bo_d = nc.dram_tensor("bo1", [1, E], f32r, kind="ExternalInput").ap()
    out_d = nc.dram_tensor("out", [T, E], f32, kind="ExternalOutput").ap()

    with tile.TileContext(nc) as tc:
        _body(nc, tc, mybir,
              x8_d, wqk_d, wv_d, wo_d, bqk_d, bv_d, bo_d, out_d,
              with_bias)

    nc.compile()
    return nc


def _body(nc, tc, mybir,
          x8_d, wqk_d, wv_d, wo_d, bqk_d, bv_d, bo_d, out_d,
          with_bias):
    from contextlib import ExitStack
    from concourse import library_config

    f32 = mybir.dt.float32
    f32r = mybir.dt.float32r
    f8 = mybir.dt.float8e4
    Exp = mybir.ActivationFunctionType.Exp
    ADD = mybir.AluOpType.add
    MULT = mybir.AluOpType.mult
    DR = mybir.MatmulPerfMode.DoubleRow
    SUB = mybir.AluOpType.subtract
    # the Vhat constant column is sqrt(E)/SZ so the normalize reciprocal
    # yields SZ/(sqrt(E)*sumexp): z lands pre-scaled by SZ for fp8 storage
    SZ = 256.0
    SQRT_E = float(np.float32(np.sqrt(E))) / SZ
    INV = 1.0 / SXW
    INVO = 1.0 / (SZ * 32.0)   # descale for the output projection copy

    ctx = ExitStack()
    with ctx:
        persist = ctx.enter_context(tc.tile_pool(name="persist", bufs=1))
        qkpool = ctx.enter_context(tc.tile_pool(name="qkpool", bufs=1))
        projp = ctx.enter_context(tc.tile_pool(name="projp", bufs=2, space="PSUM"))
        epp = ctx.enter_context(tc.tile_pool(name="epp", bufs=2, space="PSUM"))
        zp = ctx.enter_context(tc.tile_pool(name="zp", bufs=2, space="PSUM"))

        xt = []                 # [(hi tiles), (lo tiles)] per KT2
        vhat = []
        wo8 = []
        state = {}

        # ---------------- helpers ----------------
        def proj_head(b, h):
            """Q^T/K^T for (b, h): 9 DoubleRow fp8 matmuls per 512-col chunk
            (terms W8*x8 + W8*dx8 + dWlo*x8), then a descaling copy."""
            tok0 = b * N
            qk = {}
            for nm in ("q", "k"):
                qk[nm] = qkpool.tile([D, N], f32r, name=f"{nm}t", tag=f"{nm}t",
                                     bufs=2)
            for wi, nm in enumerate(("q", "k")):
                wtile = state[f"w{nm}8"]
                qt = qk[nm]
                for tc2 in range(N // 512):
                    pq = projp.tile([128, 512], f32, name="pp", tag="pp")
                    sl = slice(tok0 + tc2 * 512, tok0 + (tc2 + 1) * 512)
                    first = True
                    for wt, xterm in ((0, 0), (0, 1), (1, 0)):
                        for c in range(KT2):
                            nc.tensor.matmul(
                                pq[0:D, :],
                                wtile[:, h][:, wt][:, c],
                                xt[xterm][c][:, :, sl],
                                start=first,
                                stop=(wt == 1 and c == KT2 - 1),
                                perf_mode=DR,
                            )
                            first = False
                    if with_bias:
                        nc.vector.tensor_scalar(
                            out=qt[:, tc2 * 512:(tc2 + 1) * 512],
                            in0=pq[0:D, :],
                            scalar1=INV,
                            scalar2=state["bqk_t"][:, wi * H + h:wi * H + h + 1],
                            op0=MULT, op1=ADD,
                        )
                    else:
                        nc.vector.tensor_scalar(
                            out=qt[:, tc2 * 512:(tc2 + 1) * 512],
                            in0=pq[0:D, :],
                            scalar1=INV, scalar2=None, op0=MULT,
                        )
            return qk

        def attention(b, h, qk, zpairs, narrow=False):
            """energy -> exp -> attn@V -> normalized fp8 z split for (b, h)."""
            zT = zp.tile([128, N], f32, name="zT", tag="zT")
            for kt in range(NKT):
                ext = expp.tile([128, N], f32r, name="ext", tag="ext")
                for qc in range(2):
                    ep = epp.tile([128, 512], f32, name="ep", tag="ep")
                    nc.tensor.matmul(
                        ep,
                        qk["k"][:, kt * 128:(kt + 1) * 128],
                        qk["q"][:, qc * 512:(qc + 1) * 512],
                        start=True, stop=True,
                    )
                    nc.scalar.activation(
                        out=ext[:, qc * 512:(qc + 1) * 512], in_=ep, func=Exp)
                    nc.tensor.matmul(
                        zT[0:D + 1, qc * 512:(qc + 1) * 512],
                        vhat[b * NKT + kt][:, h, :],
                        ext[:, qc * 512:(qc + 1) * 512],
                        start=(kt == 0), stop=(kt == NKT - 1),
                    )

            # normalize: z = SZ * zT[0:D] / (sqrt(E)*sumexp), then split into
            # fp8 hi/lo for the DoubleRow output projection. Wide
            # reciprocal+broadcast for early heads (throughput); narrow
            # per-chunk chains for the close pair (latency — the output
            # projection close matmuls wait on these).
            zhi, zlo = zpairs
            pj, slot = h // 2, h % 2
            nchunk = 2 if narrow else 1
            w = N // nchunk
            sfx = "n" if narrow else "w"
            for ch in range(nchunk):
                csl = slice(ch * w, (ch + 1) * w)
                recip = rbp.tile([1, w], f32, name="recip", tag=f"recip{sfx}",
                                 bufs=2)
                nc.vector.reciprocal(out=recip, in_=zT[D:D + 1, csl])
                rb = rbp.tile([D, w], f32, name="rb", tag=f"rb{sfx}",
                              bufs=2)
                nc.gpsimd.partition_broadcast(out_ap=rb, in_ap=recip)
                for qc in range(w // 512):
                    sl = slice(ch * w + qc * 512, ch * w + (qc + 1) * 512)
                    t = rbp.tile([D, 512], f32, name="zt_t", tag="zt_t")
                    nc.vector.tensor_mul(
                        out=t, in0=zT[0:D, sl], in1=rb[:, qc * 512:(qc + 1) * 512])
                    nc.vector.tensor_copy(out=zhi[pj][:, slot, sl], in_=t)
                    nc.vector.tensor_tensor(
                        out=zlo[pj][:, slot, sl], in0=t,
                        in1=zhi[pj][:, slot, sl], op=SUB)

        def final_proj(b, zpairs, jlast):
            """fp8 DoubleRow output projection over head pairs, software-
            pipelined: the early-ready pairs of several groups are accumulated
            before the first jlast-pair matmul so the PE has work while the
            last heads' normalize chains still run."""
            zhi, zlo = zpairs
            tok0 = b * N
            groups = [(mt, half) for mt in range(NKT) for half in range(2)]
            DEPTH = 6 if b == BPC - 1 else 5
            NP = H // 2
            jopen = [j for j in range(NP) if j != jlast]
            prs = {}
            ros = {}

            def open_group(g):
                mt, half = groups[g]
                k = g % DEPTH
                if k < 2:
                    pr = projp.tile([128, 384], f32, name="pp", tag="pp")
                elif k < 4:
                    pr = epp.tile([128, 384], f32, name="fep", tag="ep")
                else:
                    pr = zp.tile([128, 384], f32, name="fzt", tag="zT")
                cols = slice(half * 384, (half + 1) * 384)
                msl = slice(mt * 128, (mt + 1) * 128)
                first = True
                for j in jopen:
                    for zt, wt in ((zhi[j], 0), (zlo[j], 0), (zhi[j], 1)):
                        nc.tensor.matmul(
                            pr, zt[:, :, msl], wo8[wt][j][:, :, cols],
                            start=first, stop=False, perf_mode=DR,
                        )
                        first = False
                prs[g] = pr

            for g in range(min(DEPTH, len(groups))):
                open_group(g)
            for g, (mt, half) in enumerate(groups):
                pr = prs.pop(g)
                cols = slice(half * 384, (half + 1) * 384)
                msl = slice(mt * 128, (mt + 1) * 128)
                j = jlast
                for ti, (zt, wt) in enumerate(
                        ((zhi[j], 0), (zlo[j], 0), (zhi[j], 1))):
                    nc.tensor.matmul(
                        pr, zt[:, :, msl], wo8[wt][j][:, :, cols],
                        start=False,
                        stop=(ti == 2 and not with_bias), perf_mode=DR,
                    )
                if with_bias:
                    # bo is pre-scaled by SZ*32 on the host
                    nc.tensor.matmul(
                        pr, onescol_r, state["bor"][:, cols],
                        start=False, stop=True,
                    )
                if half == 0:
                    ros[mt] = rop.tile([128, E], f32, name="ro", tag="ro")
                if g % 2 == 0:
                    nc.scalar.mul(out=ros[mt][:, cols], in_=pr, mul=INVO)
                else:
                    nc.vector.tensor_scalar(
                        out=ros[mt][:, cols], in0=pr,
                        scalar1=INVO, scalar2=None, op0=MULT)
                if g + DEPTH < len(groups):
                    open_group(g + DEPTH)
                # alternate the output stores across the two HWDGE queues so
                # the end-of-kernel DMA tail is not serialized on one queue
                dma_eng = nc.sync if g % 2 == 0 else nc.scalar
                dma_eng.dma_start(
                    out=out_d[tok0 + mt * 128:tok0 + (mt + 1) * 128, cols],
                    in_=ros[mt][:, cols])
                if half == 1:
                    ros.pop(mt)

        # ---------------- phase 0: loads + Vhat ----------------
        qk00 = None
        with tc.tile_pool(name="wvpool", bufs=1) as wvpool:
            for term in range(2):
                tiles = []
                for c in range(KT2):
                    tiles.append(persist.tile([128, 2, T], f8,
                                              name=f"xt{term}_{c}",
                                              tag=f"xt{term}_{c}"))
                xt.append(tiles)

            def load_x_quarter(q):
                for hf in range(2):
                    sl = slice(q * 512 + hf * 256, q * 512 + (hf + 1) * 256)
                    for term in range(2):
                        for c in range(KT2):
                            nc.sync.dma_start(
                                out=xt[term][c][:, :, sl],
                                in_=x8_d[term][:, c][:, :, sl])

            # constants
            ones_f = persist.tile([1, 128], f32, name="ones_f", tag="ones_f")
            nc.vector.memset(ones_f, 1.0)
            onescol_r = persist.tile([1, 128], f32r, name="ones_r", tag="ones_r")
            nc.vector.tensor_copy(out=onescol_r, in_=ones_f)
            c27f = persist.tile([128, 1], f32, name="c27f", tag="c27f")
            nc.vector.memset(c27f, SQRT_E)
            c27r = persist.tile([128, 1], f32r, name="c27r", tag="c27r")
            nc.vector.tensor_copy(out=c27r, in_=c27f)

            # first x quarter interleaved with Wv so the Vhat(0) psum
            # group can start accumulating early; hi terms are loaded before
            # lo terms to match the matmul emission order within each group
            wv = [[], []]
            for term in range(2):
                for c in range(KT2):
                    nc.sync.dma_start(
                        out=xt[term][c][:, :, 0:256],
                        in_=x8_d[term][:, c][:, :, 0:256])
                    wvc = wvpool.tile([128, 2, E], f8, name=f"wv{term}_{c}",
                                      tag=f"wv{term}_{c}")
                    nc.gpsimd.dma_start(out=wvc, in_=wv_d[term][:, c])
                    wv[term].append(wvc)
            for term in range(2):
                for c in range(KT2):
                    nc.sync.dma_start(
                        out=xt[term][c][:, :, 256:512],
                        in_=x8_d[term][:, c][:, :, 256:512])

            # gpsimd ucode library with partition_broadcast (needed by the
            # first normalize; emitted after the Wv loads so it does not
            # head-of-line block the gpsimd DMA queue at startup)
            nc.gpsimd.load_library(library_config.attn)

            # persistent Wq/Wk fp8 tiles (one DMA per head per q/k)
            for nm, wi in (("q", 0), ("k", 1)):
                wt8 = persist.tile([128, H, 2, KT2, 2, D], f8,
                                   name=f"w{nm}8", tag=f"w{nm}8")
                for h in range(H):
                    nc.gpsimd.dma_start(out=wt8[:, h], in_=wqk_d[wi][:, h])
                state[f"w{nm}8"] = wt8

            # biases
            bqk_t = persist.tile([D, 2 * H], f32, name="bqk_t", tag="bqk_t")
            nc.gpsimd.dma_start(out=bqk_t, in_=bqk_d)
            state["bqk_t"] = bqk_t
            bvr = persist.tile([1, E], f32r, name="bvr", tag="bvr")
            nc.gpsimd.dma_start(out=bvr, in_=bv_d)

            def build_vhat(mt):
                # Vhat[mt] : [128 tokens, H, D+1]; column D holds sqrt(E)
                vh = persist.tile([128, H, D + 1], f32r, name=f"vhat{mt}",
                                  tag=f"vhat{mt}")
                msl = slice(mt * 128, (mt + 1) * 128)
                for half in range(2):  # heads 0-3 / 4-7 (384 cols each)
                    pv = projp.tile([128, 512], f32, name="pp", tag="pp")
                    cols = slice(half * 4 * D, (half + 1) * 4 * D)
                    terms = ((0, 0), (1, 0), (0, 1))
                    for ti, (wt, xterm) in enumerate(terms):
                        for c in range(KT2):
                            nc.tensor.matmul(
                                pv[:, 0:4 * D],
                                xt[xterm][c][:, :, msl],
                                wv[wt][c][:, :, cols],
                                start=(ti == 0 and c == 0),
                                stop=(with_bias is False and ti == 2
                                      and c == KT2 - 1),
                                perf_mode=DR,
                            )
                    if with_bias:
                        # bv is pre-scaled by SXW on the host
                        nc.tensor.matmul(
                            pv[:, 0:4 * D], onescol_r, bvr[:, cols],
                            start=False, stop=True,
                        )
                    nc.scalar.mul(
                        out=vh[:, half * 4:(half + 1) * 4, 0:D],
                        in_=pv[:, 0:4 * D].rearrange("p (h d) -> p h d", h=4),
                        mul=INV,
                    )
                nc.vector.tensor_copy(
                    out=vh[:, :, D:D + 1],
                    in_=c27r.to_broadcast([128, H, 1]),
                )
                vhat.append(vh)

            # interleave: quarters 0-1 -> Vhat 0-7, then the first head
            # projection (keeps the PE busy while quarters 2-3 stream in)
            for q in range(2):
                if q > 0:
                    load_x_quarter(q)
                for mt in range(4 * q, 4 * q + 4):
                    build_vhat(mt)
            qk00 = proj_head(0, 6)
            for q in range(2, 4):
                load_x_quarter(q)
                for mt in range(4 * q, 4 * q + 4):
                    build_vhat(mt)

        # stage + wv pools released; later pools reuse their space
        expp = ctx.enter_context(tc.tile_pool(name="expp", bufs=3))
        rbp = ctx.enter_context(tc.tile_pool(name="rbp", bufs=2))
        rop = ctx.enter_context(tc.tile_pool(name="rop", bufs=2))
        ztpool = ctx.enter_context(tc.tile_pool(name="ztpool", bufs=1))
        wopool = ctx.enter_context(tc.tile_pool(name="wopool", bufs=1))

        # Wo -> fp8 hi/lo per-head-pair tiles + bo (phase 2 operands)
        for term in range(2):
            tiles = []
            for j in range(H // 2):
                woj = wopool.tile([D, 2, E], f8, name=f"wo{term}_{j}",
                                  tag=f"wo{term}_{j}")
                nc.gpsimd.dma_start(out=woj, in_=wo_d[term][:, j])
                tiles.append(woj)
            wo8.append(tiles)
        if with_bias:
            bor = wopool.tile([1, E], f32r, name="bor", tag="bor")
            nc.gpsimd.dma_start(out=bor, in_=bo_d)
            state["bor"] = bor

        # ---------------- phases 1+2, batch-major, software-pipelined ------
        # head order: pair 3 (h6,h7) first so it is ready long before the
        # output projection; pair 2 (h4,h5) finishes last and is the close
        # pair there. Each head's projection is emitted one step ahead so
        # the energy matmuls never wait on the PSUM->SBUF copy latency.
        ORDER = (6, 7, 0, 1, 2, 3, 4, 5)
        qk_next = {ORDER[0]: qk00}
        for b in range(BPC):
            zhi, zlo = [], []
            for j in range(H // 2):
                zhi.append(ztpool.tile([D, 2, N], f8, name=f"z8_{j}",
                                       tag=f"z8_{j}", bufs=2))
                zlo.append(ztpool.tile([D, 2, N], f8, name=f"dz8_{j}",
                                       tag=f"dz8_{j}", bufs=2))
            zpairs = (zhi, zlo)
            for idx, h in enumerate(ORDER):
                qk = qk_next.pop(h) if h in qk_next else proj_head(b, h)
                attention(b, h, qk, zpairs, narrow=(h // 2 == 2))
            qk_next = {}
            if b + 1 < BPC:
                # emit next batch's first projection before the output
                # projection so the PE has work while the last z normalizes
                qk_next[ORDER[0]] = proj_head(b + 1, ORDER[0])
            final_proj(b, zpairs, jlast=2)


def _get_runner(with_bias=False):
    """Build (once per variant) a jitted shard_map executing the NEFF."""
    key = ("runner", with_bias)
    if key in _CACHE:
        return _CACHE[key]

    import jax
    from jax.experimental.shard_map import shard_map
    from jax.sharding import Mesh, NamedSharding, PartitionSpec
    from concourse import mybir
    from concourse.bass2jax import (
        _bass_exec_p, install_neuronx_cc_hook, partition_id_tensor)

    nc = _build(with_bias=with_bias)
    install_neuronx_cc_hook()

    partition_name = (
        nc.partition_id_tensor.name if nc.partition_id_tensor else None)
    in_names, out_names, out_avals, zero_outs = [], [], [], []
    for alloc in nc.m.functions[0].allocations:
        if not isinstance(alloc, mybir.MemoryLocationSet):
            continue
        name = alloc.memorylocations[0].name
        if alloc.kind == "ExternalInput":
            if name != partition_name:
                in_names.append(name)
        elif alloc.kind == "ExternalOutput":
            out_names.append(name)
            shape = tuple(alloc.tensor_shape)
            dtype = mybir.dt.np(alloc.dtype)
            out_avals.append(jax.core.ShapedArray(shape, dtype))
            zero_outs.append(np.zeros(shape, dtype))
    n_params = len(in_names)
    all_in_names = in_names + out_names
    if partition_name is not None:
        all_in_names = all_in_names + [partition_name]

    def _bass_body(*args):
        operands = list(args)
        if partition_name is not None:
            operands.append(partition_id_tensor())
        outs = _bass_exec_p.bind(
            *operands,
            out_avals=tuple(out_avals),
            in_names=tuple(all_in_names),
            out_names=tuple(out_names),
            lowering_input_output_aliases=(),
            sim_require_finite=True,
            sim_require_nnan=True,
            nc=nc,
        )
        return tuple(outs)

    devices = jax.devices()[:NCORES]
    mesh = Mesh(np.asarray(devices), ("core",))
    spec = PartitionSpec("core")
    rspec = PartitionSpec()          # replicated (weights/biases)
    sharding = NamedSharding(mesh, spec)
    rsharding = NamedSharding(mesh, rspec)
    n_outs = len(out_names)
    # x8 is per-core data; everything else is identical across cores
    in_specs = tuple(spec if nm == "x8" else rspec for nm in in_names)
    jitted = jax.jit(
        shard_map(
            _bass_body, mesh=mesh,
            in_specs=in_specs + (spec,) * n_outs,
            out_specs=(spec,) * n_outs,
            check_rep=False,
        ),
        keep_unused=True,
    )
    zeros_dev = [
        jax.device_put(np.concatenate([z] * NCORES, axis=0), sharding)
        for z in zero_outs
    ]
    runner = {
        "jitted": jitted, "in_names": in_names, "out_names": out_names,
        "sharding": sharding, "rsharding": rsharding,
        "zeros_dev": zeros_dev, "jax": jax,
    }
    _CACHE[key] = runner
    return runner


def _split8(a, s):
    """fp8 residual split: return (hi, lo) e4m3 arrays with hi+lo ~= s*a."""
    import ml_dtypes
    E4 = ml_dtypes.float8_e4m3
    sa = (s * a).astype(np.float32)
    hi = sa.astype(E4)
    lo = (sa - hi.astype(np.float32)).astype(E4)
    return hi, lo


def _prep_inputs(x, Wq, bq, Wk, bk, Wv, bv, Wo, bo):
    """Host-side prep: arrays keyed by NEFF input name."""
    import ml_dtypes
    x = np.asarray(x, dtype=np.float32)
    Wq, Wk, Wv, Wo = (np.asarray(w, dtype=np.float32) for w in (Wq, Wk, Wv, Wo))
    bq, bk, bv, bo = (np.asarray(v, dtype=np.float32) for v in (bq, bk, bv, bo))

    # x: [B,N,E] -> per-core xT [E,T] -> x8[term, p, c, i, t] fp8 at scale 8
    xT = x.reshape(NCORES, T, E).transpose(0, 2, 1)     # [NC, E, T]
    xh, xl = _split8(xT, 8.0)
    # [NC, E, T] -> [NC, c, 2, 128, T] -> [NC, 128, c, 2, T]
    def xlay(a):
        return np.ascontiguousarray(
            a.reshape(NCORES, KT2, 2, 128, T).transpose(0, 3, 1, 2, 4))
    # concat over cores on the leading (sharded) axis
    x8 = np.stack([xlay(xh), xlay(xl)], axis=1).reshape(
        NCORES * 2, 128, KT2, 2, T)

    # Wq/Wk: [E, E] -> [2(qk), 128, H, 2(term), KT2, 2, D] fp8 at scale 32
    def wlay(W):
        hi, lo = _split8(W, 32.0)
        # rows k = c*256 + i*128 + p
        def lay(a):
            return a.reshape(KT2, 2, 128, H, D).transpose(2, 3, 0, 1, 4)
        # -> [128, H, KT2, 2, D]; stack terms -> [128, H, 2, KT2, 2, D]
        return np.stack([lay(hi), lay(lo)], axis=2)
    wqk = np.ascontiguousarray(np.stack([wlay(Wq), wlay(Wk)], axis=0))

    # Wv: [E, E] -> [2(term), 128, KT2, 2, E] at scale 32
    vh, vl = _split8(Wv, 32.0)
    def vlay(a):
        return a.reshape(KT2, 2, 128, E).transpose(2, 0, 1, 3)
    wv8 = np.ascontiguousarray(np.stack([vlay(vh), vlay(vl)], axis=0))

    # Wo: [E, E] -> [2(term), D, H/2(pair), 2, E] at scale 32; slot i of
    # pair j holds rows for head 2j+i
    oh, ol = _split8(Wo, 32.0)
    def olay(a):
        return a.reshape(H // 2, 2, D, E).transpose(2, 0, 1, 3)
    wo8 = np.ascontiguousarray(np.stack([olay(oh), olay(ol)], axis=0))

    bqk = np.ascontiguousarray(
        np.concatenate([bq.reshape(H, D).T, bk.reshape(H, D).T], axis=1))

    return {
        "x8": x8, "wqk8": wqk, "wv8": wv8, "wo8": wo8,
        "bqk": bqk,
        "bv1": np.ascontiguousarray(SXW * bv.reshape(1, E)),
        "bo1": np.ascontiguousarray(256.0 * 32.0 * bo.reshape(1, E)),
    }


def _run(inputs, device_resident=None, with_bias=False):
    r = _get_runner(with_bias)
    args = []
    for name in r["in_names"]:
        if device_resident is not None and name in device_resident:
            args.append(device_resident[name])
        else:
            args.append(inputs[name])
    outs = r["jitted"](*args, *r["zeros_dev"])
    return {name: outs[i] for i, name in enumerate(r["out_names"])}


def _weights_on_device(inputs, with_bias=False):
    """device_put the (replicated) weight/bias arrays once per unique value."""
    import hashlib
    r = _get_runner(with_bias)
    key = hashlib.sha1()
    for name in sorted(inputs):
        if name == "x8":
            continue
        a = inputs[name]
        key.update(name.encode())
        key.update(a.shape.__repr__().encode())
        key.update(a.tobytes())
    key = key.hexdigest()
    cached = _CACHE.get("weights_dev")
    if cached is not None and cached[0] == key:
        return cached[1]
    dev = {
        name: r["jax"].device_put(a, r["rsharding"])
        for name, a in inputs.items() if name != "x8"
    }
    _CACHE["weights_dev"] = (key, dev)
    return dev


def kernel(x, Wq, bq, Wk, bk, Wv, bv, Wo, bo):
    with_bias = any(
        np.any(np.asarray(v)) for v in (bq, bk, bv, bo))
    inputs = _prep_inputs(x, Wq, bq, Wk, bk, Wv, bv, Wo, bo)
    dev = _weights_on_device(inputs, with_bias)
    outs = _run(inputs, dev, with_bias)
    out = np.asarray(outs["out"])          # [NCORES*T, E]
    return out.reshape(B, N, E)


def bench(x, Wq, bq, Wk, bk, Wv, bv, Wo, bo, iters=20):
    """Time repeated executions with all inputs device-resident."""
    import time
    r = _get_runner()
    inputs = _prep_inputs(x, Wq, bq, Wk, bk, Wv, bv, Wo, bo)
    dev = _weights_on_device(inputs)
    dev = dict(dev)
    dev["x8"] = r["jax"].device_put(inputs["x8"], r["sharding"])

    out = _run(inputs, dev)
    list(out.values())[0].block_until_ready()

    t0 = time.time()
    last = None
    for _ in range(iters):
        last = _run(inputs, dev)
    for v in last.values():
        v.block_until_ready()
    dt = (time.time() - t0) / iters
    return dt


# revision 35
# speedup vs baseline: 1.0680x; 1.0014x over previous
#!/usr/bin/env python3
"""Multi-head attention (B=16, N=1024, E=768, H=8, softmax-then-scale variant)
as a Bass/Tile kernel on 8 TRN2 NeuronCores, data-parallel over the batch.

Per core (2 batch elements, T=2048 tokens):
  - QK projections and the V build run as fp8(e4m3) DoubleRow matmuls with a
    2-term residual split of both operands (a ~= a8 + da8 at a common scale;
    3 of the 4 cross terms are kept, the lo*lo term ~0.1% is dropped). The
    host supplies x8/dx8 at scale 8 and W8/dWlo at scale 32; the 1/256
    descale is folded into the PSUM->SBUF copies. DoubleRow contracts
    2x128 K per instruction at 0.5 cycles/row, so each term runs at 4x the
    fp32r rate and the 3-term total is 0.75x.
  - energy / attn@V / output projection stay fp32r (full-rate PE): exp
    values span e^54 so fp8 can't represent the attention weights, and the
    softmax amplifies any q/k quantization into argmax flips.
  - loop over batch b, then head h:
      energy^T per ktile: lhsT = K^T slice [96,128], rhs = Q^T [96,512]
      exp on ScalarE (no max subtraction: |energy| <~ 60 fits fp32 exp)
      attn@V flash-style: lhsT = Vhat [128, 97] (V cols for head h + a
        sqrt(E) constant column so row 96 accumulates sqrt(E)*sumexp),
        rhs = expT [128,512], accumulated over 8 k-tiles -> zT [97, 1024]
      normalize: recip = 1/zT[96] (DVE), replicated across partitions by
        the gpsimd partition_broadcast custom op, z_h = zT[0:96] * recip
    then output projection for batch b: R = sum_h z_h^T.T @ Wo_h + 1^T bo
"""
import os
import sys

sys.path.insert(0, "/opt/trn_rl_repo")

import numpy as np

B, N, E, H, D = 16, 1024, 768, 8, 96
NCORES = 8
BPC = B // NCORES          # batch elements per core
T = BPC * N                # tokens per core
KT2 = E // 256             # DoubleRow k-tiles over embedding dim (3)
MT = T // 128              # token tiles per core (16)
NKT = N // 128             # k-tiles over sequence (8)
SXW = 256.0                # x scale (8) * W scale (32)

_CACHE = {}


def _build(with_bias=True):
    import concourse.tile as tile
    from concourse import bacc, mybir

    f32 = mybir.dt.float32
    f32r = mybir.dt.float32r
    f8 = mybir.dt.float8e4

    nc = bacc.Bacc("TRN2", target_bir_lowering=False, debug=False)

    # fp8 operand pairs (hi, lo) for x, Wq/Wk, Wv; f32r elsewhere. Layouts
    # are pre-packed on the host for DoubleRow ([partition, 2, free] slices).
    x8_d = nc.dram_tensor("x8", [2, 128, KT2, 2, T], f8, kind="ExternalInput").ap()
    wqk_d = nc.dram_tensor("wqk8", [2, 128, H, 2, KT2, 2, D], f8,
                           kind="ExternalInput").ap()
    wv_d = nc.dram_tensor("wv8", [2, 128, KT2, 2, E], f8,
                          kind="ExternalInput").ap()
    wo_d = nc.dram_tensor("wo8", [2, D, H // 2, 2, E], f8,
                          kind="ExternalInput").ap()
    bqk_d = nc.dram_tensor("bqk", [D, 2 * H], f32, kind="ExternalInput").ap()
    bv_d = nc.dram_tensor("bv1", [1, E], f32r, kind="ExternalInput").ap()
    bo_d = nc.dram_tensor("bo1", [1, E], f32r, kind="ExternalInput").ap()
    out_d = nc.dram_tensor("out", [T, E], f32, kind="ExternalOutput").ap()

    with tile.TileContext(nc) as tc:
        _body(nc, tc, mybir,
              x8_d, wqk_d, wv_d, wo_d, bqk_d, bv_d, bo_d, out_d,
              with_bias)

    nc.compile()
    return nc


def _body(nc, tc, mybir,
          x8_d, wqk_d, wv_d, wo_d, bqk_d, bv_d, bo_d, out_d,
          with_bias):
    from contextlib import ExitStack
    from concourse import library_config

    f32 = mybir.dt.float32
    f32r = mybir.dt.float32r
    f8 = mybir.dt.float8e4
    Exp = mybir.ActivationFunctionType.Exp
    ADD = mybir.AluOpType.add
    MULT = mybir.AluOpType.mult
    DR = mybir.MatmulPerfMode.DoubleRow
    SUB = mybir.AluOpType.subtract
    # the Vhat constant column is sqrt(E)/SZ so the normalize reciprocal
    # yields SZ/(sqrt(E)*sumexp): z lands pre-scaled by SZ for fp8 storage
    SZ = 256.0
    SQRT_E = float(np.float32(np.sqrt(E))) / SZ
    INV = 1.0 / SXW
    INVO = 1.0 / (SZ * 32.0)   # descale for the output projection copy

    ctx = ExitStack()
    with ctx:
        persist = ctx.enter_context(tc.tile_pool(name="persist", bufs=1))
        qkpool = ctx.enter_context(tc.tile_pool(name="qkpool", bufs=1))
        projp = ctx.enter_context(tc.tile_pool(name="projp", bufs=2, space="PSUM"))
        epp = ctx.enter_context(tc.tile_pool(name="epp", bufs=2, space="PSUM"))
        zp = ctx.enter_context(tc.tile_pool(name="zp", bufs=2, space="PSUM"))

        xt = []                 # [(hi tiles), (lo tiles)] per KT2
        vhat = []
        wo8 = []
        state = {}

        # ---------------- helpers ----------------
        def proj_head(b, h):
            """Q^T/K^T for (b, h): 9 DoubleRow fp8 matmuls per 512-col chunk
            (terms W8*x8 + W8*dx8 + dWlo*x8), then a descaling copy."""
            tok0 = b * N
            qk = {}
            for nm in ("q", "k"):
                qk[nm] = qkpool.tile([D, N], f32r, name=f"{nm}t", tag=f"{nm}t",
                                     bufs=2)
            for wi, nm in enumerate(("q", "k")):
                wtile = state[f"w{nm}8"]
                qt = qk[nm]
                for tc2 in range(N // 512):
                    pq = projp.tile([128, 512], f32, name="pp", tag="pp")
                    sl = slice(tok0 + tc2 * 512, tok0 + (tc2 + 1) * 512)
                    first = True
                    for wt, xterm in ((0, 0), (0, 1), (1, 0)):
                        for c in range(KT2):
                            nc.tensor.matmul(
                                pq[0:D, :],
                                wtile[:, h][:, wt][:, c],
                                xt[xterm][c][:, :, sl],
                                start=first,
                                stop=(wt == 1 and c == KT2 - 1),
                                perf_mode=DR,
                            )
                            first = False
                    if with_bias:
                        nc.vector.tensor_scalar(
                            out=qt[:, tc2 * 512:(tc2 + 1) * 512],
                            in0=pq[0:D, :],
                            scalar1=INV,
                            scalar2=state["bqk_t"][:, wi * H + h:wi * H + h + 1],
                            op0=MULT, op1=ADD,
                        )
                    else:
                        nc.vector.tensor_scalar(
                            out=qt[:, tc2 * 512:(tc2 + 1) * 512],
                            in0=pq[0:D, :],
                            scalar1=INV, scalar2=None, op0=MULT,
                        )
            return qk

        def attention(b, h, qk, zpairs, narrow=False):
            """energy -> exp -> attn@V -> normalized fp8 z split for (b, h)."""
            zT = zp.tile([128, N], f32, name="zT", tag="zT")
            for kt in range(NKT):
                ext = expp.tile([128, N], f32r, name="ext", tag="ext")
                for qc in range(2):
                    ep = epp.tile([128, 512], f32, name="ep", tag="ep")
                    nc.tensor.matmul(
                        ep,
                        qk["k"][:, kt * 128:(kt + 1) * 128],
                        qk["q"][:, qc * 512:(qc + 1) * 512],
                        start=True, stop=True,
                    )
                    nc.scalar.activation(
                        out=ext[:, qc * 512:(qc + 1) * 512], in_=ep, func=Exp)
                    nc.tensor.matmul(
                        zT[0:D + 1, qc * 512:(qc + 1) * 512],
                        vhat[b * NKT + kt][:, h, :],
                        ext[:, qc * 512:(qc + 1) * 512],
                        start=(kt == 0), stop=(kt == NKT - 1),
                    )

            # normalize: z = SZ * zT[0:D] / (sqrt(E)*sumexp), then split into
            # fp8 hi/lo for the DoubleRow output projection. Wide
            # reciprocal+broadcast for early heads (throughput); narrow
            # per-chunk chains for the close pair (latency — the output
            # projection close matmuls wait on these).
            zhi, zlo = zpairs
            pj, slot = h // 2, h % 2
            nchunk = 2 if narrow else 1
            w = N // nchunk
            sfx = "n" if narrow else "w"
            for ch in range(nchunk):
                csl = slice(ch * w, (ch + 1) * w)
                recip = rbp.tile([1, w], f32, name="recip", tag=f"recip{sfx}",
                                 bufs=2)
                nc.vector.reciprocal(out=recip, in_=zT[D:D + 1, csl])
                rb = rbp.tile([D, w], f32, name="rb", tag=f"rb{sfx}",
                              bufs=2)
                nc.gpsimd.partition_broadcast(out_ap=rb, in_ap=recip)
                for qc in range(w // 512):
                    sl = slice(ch * w + qc * 512, ch * w + (qc + 1) * 512)
                    t = rbp.tile([D, 512], f32, name="zt_t", tag="zt_t")
                    nc.vector.tensor_mul(
                        out=t, in0=zT[0:D, sl], in1=rb[:, qc * 512:(qc + 1) * 512])
                    nc.vector.tensor_copy(out=zhi[pj][:, slot, sl], in_=t)
                    nc.vector.tensor_tensor(
                        out=zlo[pj][:, slot, sl], in0=t,
                        in1=zhi[pj][:, slot, sl], op=SUB)

        def final_proj(b, zpairs, jlast):
            """fp8 DoubleRow output projection over head pairs, software-
            pipelined: the early-ready pairs of several groups are accumulated
            before the first jlast-pair matmul so the PE has work while the
            last heads' normalize chains still run."""
            zhi, zlo = zpairs
            tok0 = b * N
            groups = [(mt, half) for mt in range(NKT) for half in range(2)]
            DEPTH = 6 if b == BPC - 1 else 5
            NP = H // 2
            jopen = [j for j in range(NP) if j != jlast]
            prs = {}
            ros = {}

            def open_group(g):
                mt, half = groups[g]
                k = g % DEPTH
                if k < 2:
                    pr = projp.tile([128, 384], f32, name="pp", tag="pp")
                elif k < 4:
                    pr = epp.tile([128, 384], f32, name="fep", tag="ep")
                else:
                    pr = zp.tile([128, 384], f32, name="fzt", tag="zT")
                cols = slice(half * 384, (half + 1) * 384)
                msl = slice(mt * 128, (mt + 1) * 128)
                first = True
                for j in jopen:
                    for zt, wt in ((zhi[j], 0), (zlo[j], 0), (zhi[j], 1)):
                        nc.tensor.matmul(
                            pr, zt[:, :, msl], wo8[wt][j][:, :, cols],
                            start=first, stop=False, perf_mode=DR,
                        )
                        first = False
                prs[g] = pr

            for g in range(min(DEPTH, len(groups))):
                open_group(g)
            for g, (mt, half) in enumerate(groups):
                pr = prs.pop(g)
                cols = slice(half * 384, (half + 1) * 384)
                msl = slice(mt * 128, (mt + 1) * 128)
                j = jlast
                for ti, (zt, wt) in enumerate(
                        ((zhi[j], 0), (zlo[j], 0), (zhi[j], 1))):
                    nc.tensor.matmul(
                        pr, zt[:, :, msl], wo8[wt][j][:, :, cols],
                        start=False,
                        stop=(ti == 2 and not with_bias), perf_mode=DR,
                    )
                if with_bias:
                    # bo is pre-scaled by SZ*32 on the host
                    nc.tensor.matmul(
                        pr, onescol_r, state["bor"][:, cols],
                        start=False, stop=True,
                    )
                if half == 0:
                    ros[mt] = rop.tile([128, E], f32, name="ro", tag="ro")
                if g % 2 == 0:
                    nc.scalar.mul(out=ros[mt][:, cols], in_=pr, mul=INVO)
                else:
                    nc.vector.tensor_scalar(
                        out=ros[mt][:, cols], in0=pr,
                        scalar1=INVO, scalar2=None, op0=MULT)
                if g + DEPTH < len(groups):
                    open_group(g + DEPTH)
                # alternate the output stores across the two HWDGE queues so
                # the end-of-kernel DMA tail is not serialized on one queue
                dma_eng = nc.sync if g % 2 == 0 else nc.scalar
                dma_eng.dma_start(
                    out=out_d[tok0 + mt * 128:tok0 + (mt + 1) * 128, cols],
                    in_=ros[mt][:, cols])
                if half == 1:
                    ros.pop(mt)

        # ---------------- phase 0: loads + Vhat ----------------
        qk00 = None
        with tc.tile_pool(name="wvpool", bufs=1) as wvpool:
            for term in range(2):
                tiles = []
                for c in range(KT2):
                    tiles.append(persist.tile([128, 2, T], f8,
                                              name=f"xt{term}_{c}",
                                              tag=f"xt{term}_{c}"))
                xt.append(tiles)

            def load_x_quarter(q):
                for hf in range(2):
                    sl = slice(q * 512 + hf * 256, q * 512 + (hf + 1) * 256)
                    for term in range(2):
                        for c in range(KT2):
                            nc.sync.dma_start(
                                out=xt[term][c][:, :, sl],
                                in_=x8_d[term][:, c][:, :, sl])

            # constants
            ones_f = persist.tile([1, 128], f32, name="ones_f", tag="ones_f")
            nc.vector.memset(ones_f, 1.0)
            onescol_r = persist.tile([1, 128], f32r, name="ones_r", tag="ones_r")
            nc.vector.tensor_copy(out=onescol_r, in_=ones_f)
            c27f = persist.tile([128, 1], f32, name="c27f", tag="c27f")
            nc.vector.memset(c27f, SQRT_E)
            c27r = persist.tile([128, 1], f32r, name="c27r", tag="c27r")
            nc.vector.tensor_copy(out=c27r, in_=c27f)

            # first x quarter interleaved with Wv so the Vhat(0) psum
            # group can start accumulating early; hi terms are loaded before
            # lo terms to match the matmul emission order within each group
            wv = [[], []]
            for term in range(2):
                for c in range(KT2):
                    nc.sync.dma_start(
                        out=xt[term][c][:, :, 0:256],
                        in_=x8_d[term][:, c][:, :, 0:256])
                    wvc = wvpool.tile([128, 2, E], f8, name=f"wv{term}_{c}",
                                      tag=f"wv{term}_{c}")
                    nc.gpsimd.dma_start(out=wvc, in_=wv_d[term][:, c])
                    wv[term].append(wvc)
            for term in range(2):
                for c in range(KT2):
                    nc.sync.dma_start(
                        out=xt[term][c][:, :, 256:512],
                        in_=x8_d[term][:, c][:, :, 256:512])

            # gpsimd ucode library with partition_broadcast (needed by the
            # first normalize; emitted after the Wv loads so it does not
            # head-of-line block the gpsimd DMA queue at startup)
            nc.gpsimd.load_library(library_config.attn)

            # persistent Wq/Wk fp8 tiles (one DMA per head per q/k)
            for nm, wi in (("q", 0), ("k", 1)):
                wt8 = persist.tile([128, H, 2, KT2, 2, D], f8,
                                   name=f"w{nm}8", tag=f"w{nm}8")
                for h in range(H):
                    nc.gpsimd.dma_start(out=wt8[:, h], in_=wqk_d[wi][:, h])
                state[f"w{nm}8"] = wt8

            # biases
            bqk_t = persist.tile([D, 2 * H], f32, name="bqk_t", tag="bqk_t")
            nc.gpsimd.dma_start(out=bqk_t, in_=bqk_d)
            state["bqk_t"] = bqk_t
            bvr = persist.tile([1, E], f32r, name="bvr", tag="bvr")
            nc.gpsimd.dma_start(out=bvr, in_=bv_d)

            def build_vhat(mt):
                # Vhat[mt] : [128 tokens, H, D+1]; column D holds sqrt(E)
                vh = persist.tile([128, H, D + 1], f32r, name=f"vhat{mt}",
                                  tag=f"vhat{mt}")
                msl = slice(mt * 128, (mt + 1) * 128)
                for half in range(2):  # heads 0-3 / 4-7 (384 cols each)
                    pv = projp.tile([128, 512], f32, name="pp", tag="pp")
                    cols = slice(half * 4 * D, (half + 1) * 4 * D)
                    terms = ((0, 0), (1, 0), (0, 1))
                    for ti, (wt, xterm) in enumerate(terms):
                        for c in range(KT2):
                            nc.tensor.matmul(
                                pv[:, 0:4 * D],
                                xt[xterm][c][:, :, msl],
                                wv[wt][c][:, :, cols],
                                start=(ti == 0 and c == 0),
                                stop=(with_bias is False and ti == 2
                                      and c == KT2 - 1),
                                perf_mode=DR,
                            )
                    if with_bias:
                        # bv is pre-scaled by SXW on the host
                        nc.tensor.matmul(
                            pv[:, 0:4 * D], onescol_r, bvr[:, cols],
                            start=False, stop=True,
                        )
                    if mt < 8:
                        nc.scalar.mul(
                            out=vh[:, half * 4:(half + 1) * 4, 0:D],
                            in_=pv[:, 0:4 * D].rearrange("p (h d) -> p h d",
                                                         h=4),
                            mul=INV,
                        )
                    else:
                        # later Vhat copies on DVE: on ACT they would queue
                        # ahead of the first attention exps
                        nc.vector.tensor_scalar(
                            out=vh[:, half * 4:(half + 1) * 4, 0:D],
                            in0=pv[:, 0:4 * D].rearrange("p (h d) -> p h d",
                                                         h=4),
                            scalar1=INV, scalar2=None, op0=MULT,
                        )
                nc.vector.tensor_copy(
                    out=vh[:, :, D:D + 1],
                    in_=c27r.to_broadcast([128, H, 1]),
                )
                vhat.append(vh)

            # interleave: quarters 0-1 -> Vhat 0-7, then the first head
            # projection (keeps the PE busy while quarters 2-3 stream in)
            for q in range(2):
                if q > 0:
                    load_x_quarter(q)
                for mt in range(4 * q, 4 * q + 4):
                    build_vhat(mt)
            qk00 = proj_head(0, 6)
            for q in range(2, 4):
                load_x_quarter(q)
                for mt in range(4 * q, 4 * q + 4):
                    build_vhat(mt)

        # stage + wv pools released; later pools reuse their space
        expp = ctx.enter_context(tc.tile_pool(name="expp", bufs=3))
        rbp = ctx.enter_context(tc.tile_pool(name="rbp", bufs=2))
        rop = ctx.enter_context(tc.tile_pool(name="rop", bufs=2))
        ztpool = ctx.enter_context(tc.tile_pool(name="ztpool", bufs=1))
        wopool = ctx.enter_context(tc.tile_pool(name="wopool", bufs=1))

        # Wo -> fp8 hi/lo per-head-pair tiles + bo (phase 2 operands)
        for term in range(2):
            tiles = []
            for j in range(H // 2):
                woj = wopool.tile([D, 2, E], f8, name=f"wo{term}_{j}",
                                  tag=f"wo{term}_{j}")
                nc.gpsimd.dma_start(out=woj, in_=wo_d[term][:, j])
                tiles.append(woj)
            wo8.append(tiles)
        if with_bias:
            bor = wopool.tile([1, E], f32r, name="bor", tag="bor")
            nc.gpsimd.dma_start(out=bor, in_=bo_d)
            state["bor"] = bor

        # ---------------- phases 1+2, batch-major, software-pipelined ------
        # head order: pair 3 (h6,h7) first so it is ready long before the
        # output projection; pair 2 (h4,h5) finishes last and is the close
        # pair there. Each head's projection is emitted one step ahead so
        # the energy matmuls never wait on the PSUM->SBUF copy latency.
        ORDER = (6, 7, 0, 1, 2, 3, 4, 5)
        qk_next = {ORDER[0]: qk00}
        for b in range(BPC):
            zhi, zlo = [], []
            for j in range(H // 2):
                zhi.append(ztpool.tile([D, 2, N], f8, name=f"z8_{j}",
                                       tag=f"z8_{j}", bufs=2))
                zlo.append(ztpool.tile([D, 2, N], f8, name=f"dz8_{j}",
                                       tag=f"dz8_{j}", bufs=2))
            zpairs = (zhi, zlo)
            for idx, h in enumerate(ORDER):
                qk = qk_next.pop(h) if h in qk_next else proj_head(b, h)
                attention(b, h, qk, zpairs, narrow=(h // 2 == 2))
            qk_next = {}
            if b + 1 < BPC:
                # emit next batch's first projection before the output
                # projection so the PE has work while the last z normalizes
                qk_next[ORDER[0]] = proj_head(b + 1, ORDER[0])
            final_proj(b, zpairs, jlast=2)


def _get_runner(with_bias=False):
    """Build (once per variant) a jitted shard_map executing the NEFF."""
    key = ("runner", with_bias)
    if key in _CACHE:
        return _CACHE[key]

    import jax
    from jax.experimental.shard_map import shard_map
    from jax.sharding import Mesh, NamedSharding, PartitionSpec
    from concourse import mybir
    from concourse.bass2jax import (
        _bass_exec_p, install_neuronx_cc_hook, partition_id_tensor)

    nc = _build(with_bias=with_bias)
    install_neuronx_cc_hook()

    partition_name = (
        nc.partition_id_tensor.name if nc.partition_id_tensor else None)
    in_names, out_names, out_avals, zero_outs = [], [], [], []
    for alloc in nc.m.functions[0].allocations:
        if not isinstance(alloc, mybir.MemoryLocationSet):
            continue
        name = alloc.memorylocations[0].name
        if alloc.kind == "ExternalInput":
            if name != partition_name:
                in_names.append(name)
        elif alloc.kind == "ExternalOutput":
            out_names.append(name)
            shape = tuple(alloc.tensor_shape)
            dtype = mybir.dt.np(alloc.dtype)
            out_avals.append(jax.core.ShapedArray(shape, dtype))
            zero_outs.append(np.zeros(shape, dtype))
    n_params = len(in_names)
    all_in_names = in_names + out_names
    if partition_name is not None:
        all_in_names = all_in_names + [partition_name]

    def _bass_body(*args):
        operands = list(args)
        if partition_name is not None:
            operands.append(partition_id_tensor())
        outs = _bass_exec_p.bind(
            *operands,
            out_avals=tuple(out_avals),
            in_names=tuple(all_in_names),
            out_names=tuple(out_names),
            lowering_input_output_aliases=(),
            sim_require_finite=True,
            sim_require_nnan=True,
            nc=nc,
        )
        return tuple(outs)

    devices = jax.devices()[:NCORES]
    mesh = Mesh(np.asarray(devices), ("core",))
    spec = PartitionSpec("core")
    rspec = PartitionSpec()          # replicated (weights/biases)
    sharding = NamedSharding(mesh, spec)
    rsharding = NamedSharding(mesh, rspec)
    n_outs = len(out_names)
    # x8 is per-core data; everything else is identical across cores
    in_specs = tuple(spec if nm == "x8" else rspec for nm in in_names)
    jitted = jax.jit(
        shard_map(
            _bass_body, mesh=mesh,
            in_specs=in_specs + (spec,) * n_outs,
            out_specs=(spec,) * n_outs,
            check_rep=False,
        ),
        keep_unused=True,
    )
    zeros_dev = [
        jax.device_put(np.concatenate([z] * NCORES, axis=0), sharding)
        for z in zero_outs
    ]
    runner = {
        "jitted": jitted, "in_names": in_names, "out_names": out_names,
        "sharding": sharding, "rsharding": rsharding,
        "zeros_dev": zeros_dev, "jax": jax,
    }
    _CACHE[key] = runner
    return runner


def _split8(a, s):
    """fp8 residual split: return (hi, lo) e4m3 arrays with hi+lo ~= s*a."""
    import ml_dtypes
    E4 = ml_dtypes.float8_e4m3
    sa = (s * a).astype(np.float32)
    hi = sa.astype(E4)
    lo = (sa - hi.astype(np.float32)).astype(E4)
    return hi, lo


def _prep_inputs(x, Wq, bq, Wk, bk, Wv, bv, Wo, bo):
    """Host-side prep: arrays keyed by NEFF input name."""
    import ml_dtypes
    x = np.asarray(x, dtype=np.float32)
    Wq, Wk, Wv, Wo = (np.asarray(w, dtype=np.float32) for w in (Wq, Wk, Wv, Wo))
    bq, bk, bv, bo = (np.asarray(v, dtype=np.float32) for v in (bq, bk, bv, bo))

    # x: [B,N,E] -> per-core xT [E,T] -> x8[term, p, c, i, t] fp8 at scale 8
    xT = x.reshape(NCORES, T, E).transpose(0, 2, 1)     # [NC, E, T]
    xh, xl = _split8(xT, 8.0)
    # [NC, E, T] -> [NC, c, 2, 128, T] -> [NC, 128, c, 2, T]
    def xlay(a):
        return np.ascontiguousarray(
            a.reshape(NCORES, KT2, 2, 128, T).transpose(0, 3, 1, 2, 4))
    # concat over cores on the leading (sharded) axis
    x8 = np.stack([xlay(xh), xlay(xl)], axis=1).reshape(
        NCORES * 2, 128, KT2, 2, T)

    # Wq/Wk: [E, E] -> [2(qk), 128, H, 2(term), KT2, 2, D] fp8 at scale 32
    def wlay(W):
        hi, lo = _split8(W, 32.0)
        # rows k = c*256 + i*128 + p
        def lay(a):
            return a.reshape(KT2, 2, 128, H, D).transpose(2, 3, 0, 1, 4)
        # -> [128, H, KT2, 2, D]; stack terms -> [128, H, 2, KT2, 2, D]
        return np.stack([lay(hi), lay(lo)], axis=2)
    wqk = np.ascontiguousarray(np.stack([wlay(Wq), wlay(Wk)], axis=0))

    # Wv: [E, E] -> [2(term), 128, KT2, 2, E] at scale 32
    vh, vl = _split8(Wv, 32.0)
    def vlay(a):
        return a.reshape(KT2, 2, 128, E).transpose(2, 0, 1, 3)
    wv8 = np.ascontiguousarray(np.stack([vlay(vh), vlay(vl)], axis=0))

    # Wo: [E, E] -> [2(term), D, H/2(pair), 2, E] at scale 32; slot i of
    # pair j holds rows for head 2j+i
    oh, ol = _split8(Wo, 32.0)
    def olay(a):
        return a.reshape(H // 2, 2, D, E).transpose(2, 0, 1, 3)
    wo8 = np.ascontiguousarray(np.stack([olay(oh), olay(ol)], axis=0))

    bqk = np.ascontiguousarray(
        np.concatenate([bq.reshape(H, D).T, bk.reshape(H, D).T], axis=1))

    return {
        "x8": x8, "wqk8": wqk, "wv8": wv8, "wo8": wo8,
        "bqk": bqk,
        "bv1": np.ascontiguousarray(SXW * bv.reshape(1, E)),
        "bo1": np.ascontiguousarray(256.0 * 32.0 * bo.reshape(1, E)),
    }


def _run(inputs, device_resident=None, with_bias=False):
    r = _get_runner(with_bias)
    args = []
    for name in r["in_names"]:
        if device_resident is not None and name in device_resident:
            args.append(device_resident[name])
        else:
            args.append(inputs[name])
    outs = r["jitted"](*args, *r["zeros_dev"])
    return {name: outs[i] for i, name in enumerate(r["out_names"])}


def _weights_on_device(inputs, with_bias=False):
    """device_put the (replicated) weight/bias arrays once per unique value."""
    import hashlib
    r = _get_runner(with_bias)
    key = hashlib.sha1()
    for name in sorted(inputs):
        if name == "x8":
            continue
        a = inputs[name]
        key.update(name.encode())
        key.update(a.shape.__repr__().encode())
        key.update(a.tobytes())
    key = key.hexdigest()
    cached = _CACHE.get("weights_dev")
    if cached is not None and cached[0] == key:
        return cached[1]
    dev = {
        name: r["jax"].device_put(a, r["rsharding"])
        for name, a in inputs.items() if name != "x8"
    }
    _CACHE["weights_dev"] = (key, dev)
    return dev


def kernel(x, Wq, bq, Wk, bk, Wv, bv, Wo, bo):
    with_bias = any(
        np.any(np.asarray(v)) for v in (bq, bk, bv, bo))
    inputs = _prep_inputs(x, Wq, bq, Wk, bk, Wv, bv, Wo, bo)
    dev = _weights_on_device(inputs, with_bias)
    outs = _run(inputs, dev, with_bias)
    out = np.asarray(outs["out"])          # [NCORES*T, E]
    return out.reshape(B, N, E)


def bench(x, Wq, bq, Wk, bk, Wv, bv, Wo, bo, iters=20):
    """Time repeated executions with all inputs device-resident."""
    import time
    r = _get_runner()
    inputs = _prep_inputs(x, Wq, bq, Wk, bk, Wv, bv, Wo, bo)
    dev = _weights_on_device(inputs)
    dev = dict(dev)
    dev["x8"] = r["jax"].device_put(inputs["x8"], r["sharding"])

    out = _run(inputs, dev)
    list(out.values())[0].block_until_ready()

    t0 = time.time()
    last = None
    for _ in range(iters):
        last = _run(inputs, dev)
    for v in last.values():
        v.block_until_ready()
    dt = (time.time() - t0) / iters
    return dt


# revision 36
# speedup vs baseline: 1.0712x; 1.0030x over previous
#!/usr/bin/env python3
"""Multi-head attention (B=16, N=1024, E=768, H=8, softmax-then-scale variant)
as a Bass/Tile kernel on 8 TRN2 NeuronCores, data-parallel over the batch.

Per core (2 batch elements, T=2048 tokens):
  - QK projections and the V build run as fp8(e4m3) DoubleRow matmuls with a
    2-term residual split of both operands (a ~= a8 + da8 at a common scale;
    3 of the 4 cross terms are kept, the lo*lo term ~0.1% is dropped). The
    host supplies x8/dx8 at scale 8 and W8/dWlo at scale 32; the 1/256
    descale is folded into the PSUM->SBUF copies. DoubleRow contracts
    2x128 K per instruction at 0.5 cycles/row, so each term runs at 4x the
    fp32r rate and the 3-term total is 0.75x.
  - energy / attn@V / output projection stay fp32r (full-rate PE): exp
    values span e^54 so fp8 can't represent the attention weights, and the
    softmax amplifies any q/k quantization into argmax flips.
  - loop over batch b, then head h:
      energy^T per ktile: lhsT = K^T slice [96,128], rhs = Q^T [96,512]
      exp on ScalarE (no max subtraction: |energy| <~ 60 fits fp32 exp)
      attn@V flash-style: lhsT = Vhat [128, 97] (V cols for head h + a
        sqrt(E) constant column so row 96 accumulates sqrt(E)*sumexp),
        rhs = expT [128,512], accumulated over 8 k-tiles -> zT [97, 1024]
      normalize: recip = 1/zT[96] (DVE), replicated across partitions by
        the gpsimd partition_broadcast custom op, z_h = zT[0:96] * recip
    then output projection for batch b: R = sum_h z_h^T.T @ Wo_h + 1^T bo
"""
import os
import sys

sys.path.insert(0, "/opt/trn_rl_repo")

import numpy as np

B, N, E, H, D = 16, 1024, 768, 8, 96
NCORES = 8
BPC = B // NCORES          # batch elements per core
T = BPC * N                # tokens per core
KT2 = E // 256             # DoubleRow k-tiles over embedding dim (3)
MT = T // 128              # token tiles per core (16)
NKT = N // 128             # k-tiles over sequence (8)
SXW = 256.0                # x scale (8) * W scale (32)

_CACHE = {}


def _build(with_bias=True):
    import concourse.tile as tile
    from concourse import bacc, mybir

    f32 = mybir.dt.float32
    f32r = mybir.dt.float32r
    f8 = mybir.dt.float8e4

    nc = bacc.Bacc("TRN2", target_bir_lowering=False, debug=False)

    # fp8 operand pairs (hi, lo) for x, Wq/Wk, Wv; f32r elsewhere. Layouts
    # are pre-packed on the host for DoubleRow ([partition, 2, free] slices).
    x8_d = nc.dram_tensor("x8", [2, 128, KT2, 2, T], f8, kind="ExternalInput").ap()
    wqk_d = nc.dram_tensor("wqk8", [2, 128, H, 2, KT2, 2, D], f8,
                           kind="ExternalInput").ap()
    wv_d = nc.dram_tensor("wv8", [2, 128, KT2, 2, E], f8,
                          kind="ExternalInput").ap()
    wo_d = nc.dram_tensor("wo8", [2, D, H // 2, 2, E], f8,
                          kind="ExternalInput").ap()
    bqk_d = nc.dram_tensor("bqk", [D, 2 * H], f32, kind="ExternalInput").ap()
    bv_d = nc.dram_tensor("bv1", [1, E], f32r, kind="ExternalInput").ap()
    bo_d = nc.dram_tensor("bo1", [1, E], f32r, kind="ExternalInput").ap()
    out_d = nc.dram_tensor("out", [T, E], f32, kind="ExternalOutput").ap()

    with tile.TileContext(nc) as tc:
        _body(nc, tc, mybir,
              x8_d, wqk_d, wv_d, wo_d, bqk_d, bv_d, bo_d, out_d,
              with_bias)

    nc.compile()
    return nc


def _body(nc, tc, mybir,
          x8_d, wqk_d, wv_d, wo_d, bqk_d, bv_d, bo_d, out_d,
          with_bias):
    from contextlib import ExitStack
    from concourse import library_config

    f32 = mybir.dt.float32
    f32r = mybir.dt.float32r
    f8 = mybir.dt.float8e4
    Exp = mybir.ActivationFunctionType.Exp
    ADD = mybir.AluOpType.add
    MULT = mybir.AluOpType.mult
    DR = mybir.MatmulPerfMode.DoubleRow
    SUB = mybir.AluOpType.subtract
    # the Vhat constant column is sqrt(E)/SZ so the normalize reciprocal
    # yields SZ/(sqrt(E)*sumexp): z lands pre-scaled by SZ for fp8 storage
    SZ = 256.0
    SQRT_E = float(np.float32(np.sqrt(E))) / SZ
    INV = 1.0 / SXW
    INVO = 1.0 / (SZ * 32.0)   # descale for the output projection copy

    ctx = ExitStack()
    with ctx:
        persist = ctx.enter_context(tc.tile_pool(name="persist", bufs=1))
        qkpool = ctx.enter_context(tc.tile_pool(name="qkpool", bufs=1))
        projp = ctx.enter_context(tc.tile_pool(name="projp", bufs=2, space="PSUM"))
        epp = ctx.enter_context(tc.tile_pool(name="epp", bufs=2, space="PSUM"))
        zp = ctx.enter_context(tc.tile_pool(name="zp", bufs=2, space="PSUM"))

        xt = []                 # [(hi tiles), (lo tiles)] per KT2
        vhat = []
        wo8 = []
        state = {}

        # ---------------- helpers ----------------
        def proj_head(b, h):
            """Q^T/K^T for (b, h): 9 DoubleRow fp8 matmuls per 512-col chunk
            (terms W8*x8 + W8*dx8 + dWlo*x8), then a descaling copy."""
            tok0 = b * N
            qk = {}
            for nm in ("q", "k"):
                qk[nm] = qkpool.tile([D, N], f32r, name=f"{nm}t", tag=f"{nm}t",
                                     bufs=2)
            for wi, nm in enumerate(("q", "k")):
                wtile = state[f"w{nm}8"]
                qt = qk[nm]
                for tc2 in range(N // 512):
                    pq = projp.tile([128, 512], f32, name="pp", tag="pp")
                    sl = slice(tok0 + tc2 * 512, tok0 + (tc2 + 1) * 512)
                    first = True
                    for wt, xterm in ((0, 0), (0, 1), (1, 0)):
                        for c in range(KT2):
                            nc.tensor.matmul(
                                pq[0:D, :],
                                wtile[:, h][:, wt][:, c],
                                xt[xterm][c][:, :, sl],
                                start=first,
                                stop=(wt == 1 and c == KT2 - 1),
                                perf_mode=DR,
                            )
                            first = False
                    if with_bias:
                        nc.vector.tensor_scalar(
                            out=qt[:, tc2 * 512:(tc2 + 1) * 512],
                            in0=pq[0:D, :],
                            scalar1=INV,
                            scalar2=state["bqk_t"][:, wi * H + h:wi * H + h + 1],
                            op0=MULT, op1=ADD,
                        )
                    else:
                        nc.vector.tensor_scalar(
                            out=qt[:, tc2 * 512:(tc2 + 1) * 512],
                            in0=pq[0:D, :],
                            scalar1=INV, scalar2=None, op0=MULT,
                        )
            return qk

        def attention(b, h, qk, zpairs, narrow=False):
            """energy -> exp -> attn@V -> normalized fp8 z split for (b, h)."""
            zT = zp.tile([128, N], f32, name="zT", tag="zT")
            for kt in range(NKT):
                ext = expp.tile([128, N], f32r, name="ext", tag="ext")
                for qc in range(2):
                    ep = epp.tile([128, 512], f32, name="ep", tag="ep")
                    nc.tensor.matmul(
                        ep,
                        qk["k"][:, kt * 128:(kt + 1) * 128],
                        qk["q"][:, qc * 512:(qc + 1) * 512],
                        start=True, stop=True,
                    )
                    nc.scalar.activation(
                        out=ext[:, qc * 512:(qc + 1) * 512], in_=ep, func=Exp)
                    nc.tensor.matmul(
                        zT[0:D + 1, qc * 512:(qc + 1) * 512],
                        vhat[b * NKT + kt][:, h, :],
                        ext[:, qc * 512:(qc + 1) * 512],
                        start=(kt == 0), stop=(kt == NKT - 1),
                    )

            # normalize: z = SZ * zT[0:D] / (sqrt(E)*sumexp), then split into
            # fp8 hi/lo for the DoubleRow output projection. Wide
            # reciprocal+broadcast for early heads (throughput); narrow
            # per-chunk chains for the close pair (latency — the output
            # projection close matmuls wait on these).
            zhi, zlo = zpairs
            pj, slot = h // 2, h % 2
            nchunk = 2 if narrow else 1
            w = N // nchunk
            sfx = "n" if narrow else "w"
            for ch in range(nchunk):
                csl = slice(ch * w, (ch + 1) * w)
                recip = rbp.tile([1, w], f32, name="recip", tag=f"recip{sfx}",
                                 bufs=2)
                nc.vector.reciprocal(out=recip, in_=zT[D:D + 1, csl])
                rb = rbp.tile([D, w], f32, name="rb", tag=f"rb{sfx}",
                              bufs=2)
                nc.gpsimd.partition_broadcast(out_ap=rb, in_ap=recip)
                for qc in range(w // 512):
                    sl = slice(ch * w + qc * 512, ch * w + (qc + 1) * 512)
                    t = rbp.tile([D, 512], f32, name="zt_t", tag="zt_t")
                    nc.vector.tensor_mul(
                        out=t, in0=zT[0:D, sl], in1=rb[:, qc * 512:(qc + 1) * 512])
                    nc.vector.tensor_copy(out=zhi[pj][:, slot, sl], in_=t)
                    nc.vector.tensor_tensor(
                        out=zlo[pj][:, slot, sl], in0=t,
                        in1=zhi[pj][:, slot, sl], op=SUB)

        def final_proj(b, zpairs, jlast):
            """fp8 DoubleRow output projection over head pairs, software-
            pipelined: the early-ready pairs of several groups are accumulated
            before the first jlast-pair matmul so the PE has work while the
            last heads' normalize chains still run."""
            zhi, zlo = zpairs
            tok0 = b * N
            groups = [(mt, half) for mt in range(NKT) for half in range(2)]
            DEPTH = 6 if b == BPC - 1 else 5
            NP = H // 2
            jopen = [j for j in range(NP) if j != jlast]
            prs = {}
            ros = {}

            def open_group(g):
                mt, half = groups[g]
                k = g % DEPTH
                if k < 2:
                    pr = projp.tile([128, 384], f32, name="pp", tag="pp")
                elif k < 4:
                    pr = epp.tile([128, 384], f32, name="fep", tag="ep")
                else:
                    pr = zp.tile([128, 384], f32, name="fzt", tag="zT")
                cols = slice(half * 384, (half + 1) * 384)
                msl = slice(mt * 128, (mt + 1) * 128)
                first = True
                for j in jopen:
                    for zt, wt in ((zhi[j], 0), (zlo[j], 0), (zhi[j], 1)):
                        nc.tensor.matmul(
                            pr, zt[:, :, msl], wo8[wt][j][:, :, cols],
                            start=first, stop=False, perf_mode=DR,
                        )
                        first = False
                prs[g] = pr

            for g in range(min(DEPTH, len(groups))):
                open_group(g)
            for g, (mt, half) in enumerate(groups):
                pr = prs.pop(g)
                cols = slice(half * 384, (half + 1) * 384)
                msl = slice(mt * 128, (mt + 1) * 128)
                j = jlast
                for ti, (zt, wt) in enumerate(
                        ((zhi[j], 0), (zlo[j], 0), (zhi[j], 1))):
                    nc.tensor.matmul(
                        pr, zt[:, :, msl], wo8[wt][j][:, :, cols],
                        start=False,
                        stop=(ti == 2 and not with_bias), perf_mode=DR,
                    )
                if with_bias:
                    # bo is pre-scaled by SZ*32 on the host
                    nc.tensor.matmul(
                        pr, onescol_r, state["bor"][:, cols],
                        start=False, stop=True,
                    )
                if half == 0:
                    ros[mt] = rop.tile([128, E], f32, name="ro", tag="ro")
                if g % 2 == 0:
                    nc.scalar.mul(out=ros[mt][:, cols], in_=pr, mul=INVO)
                else:
                    nc.vector.tensor_scalar(
                        out=ros[mt][:, cols], in0=pr,
                        scalar1=INVO, scalar2=None, op0=MULT)
                if g + DEPTH < len(groups):
                    open_group(g + DEPTH)
                # alternate the output stores across the two HWDGE queues so
                # the end-of-kernel DMA tail is not serialized on one queue
                dma_eng = nc.sync if g % 2 == 0 else nc.scalar
                dma_eng.dma_start(
                    out=out_d[tok0 + mt * 128:tok0 + (mt + 1) * 128, cols],
                    in_=ros[mt][:, cols])
                if half == 1:
                    ros.pop(mt)

        # ---------------- phase 0: loads + Vhat ----------------
        qk00 = None
        with tc.tile_pool(name="wvpool", bufs=1) as wvpool:
            for term in range(2):
                tiles = []
                for c in range(KT2):
                    tiles.append(persist.tile([128, 2, T], f8,
                                              name=f"xt{term}_{c}",
                                              tag=f"xt{term}_{c}"))
                xt.append(tiles)

            def load_x_quarter(q):
                for hf in range(2):
                    sl = slice(q * 512 + hf * 256, q * 512 + (hf + 1) * 256)
                    for term in range(2):
                        for c in range(KT2):
                            nc.sync.dma_start(
                                out=xt[term][c][:, :, sl],
                                in_=x8_d[term][:, c][:, :, sl])

            # constants
            ones_f = persist.tile([1, 128], f32, name="ones_f", tag="ones_f")
            nc.vector.memset(ones_f, 1.0)
            onescol_r = persist.tile([1, 128], f32r, name="ones_r", tag="ones_r")
            nc.vector.tensor_copy(out=onescol_r, in_=ones_f)
            c27f = persist.tile([128, 1], f32, name="c27f", tag="c27f")
            nc.vector.memset(c27f, SQRT_E)
            c27r = persist.tile([128, 1], f32r, name="c27r", tag="c27r")
            nc.vector.tensor_copy(out=c27r, in_=c27f)

            # first x quarter interleaved with Wv so the Vhat(0) psum
            # group can start accumulating early; hi terms are loaded before
            # lo terms to match the matmul emission order within each group
            wv = [[], []]
            for term in range(2):
                for c in range(KT2):
                    nc.sync.dma_start(
                        out=xt[term][c][:, :, 0:256],
                        in_=x8_d[term][:, c][:, :, 0:256])
                    wvc = wvpool.tile([128, 2, E], f8, name=f"wv{term}_{c}",
                                      tag=f"wv{term}_{c}")
                    nc.gpsimd.dma_start(out=wvc, in_=wv_d[term][:, c])
                    wv[term].append(wvc)
            for term in range(2):
                for c in range(KT2):
                    nc.sync.dma_start(
                        out=xt[term][c][:, :, 256:512],
                        in_=x8_d[term][:, c][:, :, 256:512])

            # gpsimd ucode library with partition_broadcast (needed by the
            # first normalize; emitted after the Wv loads so it does not
            # head-of-line block the gpsimd DMA queue at startup)
            nc.gpsimd.load_library(library_config.attn)

            # persistent Wq/Wk fp8 tiles (one DMA per head per q/k)
            for nm, wi in (("q", 0), ("k", 1)):
                wt8 = persist.tile([128, H, 2, KT2, 2, D], f8,
                                   name=f"w{nm}8", tag=f"w{nm}8")
                for h in (6, 7, 0, 1, 2, 3, 4, 5):
                    nc.gpsimd.dma_start(out=wt8[:, h], in_=wqk_d[wi][:, h])
                state[f"w{nm}8"] = wt8

            # biases
            bqk_t = persist.tile([D, 2 * H], f32, name="bqk_t", tag="bqk_t")
            nc.gpsimd.dma_start(out=bqk_t, in_=bqk_d)
            state["bqk_t"] = bqk_t
            bvr = persist.tile([1, E], f32r, name="bvr", tag="bvr")
            nc.gpsimd.dma_start(out=bvr, in_=bv_d)

            def build_vhat(mt):
                # Vhat[mt] : [128 tokens, H, D+1]; column D holds sqrt(E)
                vh = persist.tile([128, H, D + 1], f32r, name=f"vhat{mt}",
                                  tag=f"vhat{mt}")
                msl = slice(mt * 128, (mt + 1) * 128)
                for half in range(2):  # heads 0-3 / 4-7 (384 cols each)
                    pv = projp.tile([128, 512], f32, name="pp", tag="pp")
                    cols = slice(half * 4 * D, (half + 1) * 4 * D)
                    terms = ((0, 0), (1, 0), (0, 1))
                    for ti, (wt, xterm) in enumerate(terms):
                        for c in range(KT2):
                            nc.tensor.matmul(
                                pv[:, 0:4 * D],
                                xt[xterm][c][:, :, msl],
                                wv[wt][c][:, :, cols],
                                start=(ti == 0 and c == 0),
                                stop=(with_bias is False and ti == 2
                                      and c == KT2 - 1),
                                perf_mode=DR,
                            )
                    if with_bias:
                        # bv is pre-scaled by SXW on the host
                        nc.tensor.matmul(
                            pv[:, 0:4 * D], onescol_r, bvr[:, cols],
                            start=False, stop=True,
                        )
                    if mt < 8:
                        nc.scalar.mul(
                            out=vh[:, half * 4:(half + 1) * 4, 0:D],
                            in_=pv[:, 0:4 * D].rearrange("p (h d) -> p h d",
                                                         h=4),
                            mul=INV,
                        )
                    else:
                        # later Vhat copies on DVE: on ACT they would queue
                        # ahead of the first attention exps
                        nc.vector.tensor_scalar(
                            out=vh[:, half * 4:(half + 1) * 4, 0:D],
                            in0=pv[:, 0:4 * D].rearrange("p (h d) -> p h d",
                                                         h=4),
                            scalar1=INV, scalar2=None, op0=MULT,
                        )
                nc.vector.tensor_copy(
                    out=vh[:, :, D:D + 1],
                    in_=c27r.to_broadcast([128, H, 1]),
                )
                vhat.append(vh)

            # interleave: quarters 0-1 -> Vhat 0-7, then the first head
            # projection (keeps the PE busy while quarters 2-3 stream in)
            for q in range(2):
                if q > 0:
                    load_x_quarter(q)
                for mt in range(4 * q, 4 * q + 4):
                    build_vhat(mt)
            qk00 = proj_head(0, 6)
            for q in range(2, 4):
                load_x_quarter(q)
                for mt in range(4 * q, 4 * q + 4):
                    build_vhat(mt)

        # stage + wv pools released; later pools reuse their space
        expp = ctx.enter_context(tc.tile_pool(name="expp", bufs=3))
        rbp = ctx.enter_context(tc.tile_pool(name="rbp", bufs=2))
        rop = ctx.enter_context(tc.tile_pool(name="rop", bufs=2))
        ztpool = ctx.enter_context(tc.tile_pool(name="ztpool", bufs=1))
        wopool = ctx.enter_context(tc.tile_pool(name="wopool", bufs=1))

        # Wo -> fp8 hi/lo per-head-pair tiles + bo (phase 2 operands)
        for term in range(2):
            tiles = []
            for j in range(H // 2):
                woj = wopool.tile([D, 2, E], f8, name=f"wo{term}_{j}",
                                  tag=f"wo{term}_{j}")
                nc.gpsimd.dma_start(out=woj, in_=wo_d[term][:, j])
                tiles.append(woj)
            wo8.append(tiles)
        if with_bias:
            bor = wopool.tile([1, E], f32r, name="bor", tag="bor")
            nc.gpsimd.dma_start(out=bor, in_=bo_d)
            state["bor"] = bor

        # ---------------- phases 1+2, batch-major, software-pipelined ------
        # head order: pair 3 (h6,h7) first so it is ready long before the
        # output projection; pair 2 (h4,h5) finishes last and is the close
        # pair there. Each head's projection is emitted one step ahead so
        # the energy matmuls never wait on the PSUM->SBUF copy latency.
        ORDER = (6, 7, 0, 1, 2, 3, 4, 5)
        qk_next = {ORDER[0]: qk00}
        for b in range(BPC):
            zhi, zlo = [], []
            for j in range(H // 2):
                zhi.append(ztpool.tile([D, 2, N], f8, name=f"z8_{j}",
                                       tag=f"z8_{j}", bufs=2))
                zlo.append(ztpool.tile([D, 2, N], f8, name=f"dz8_{j}",
                                       tag=f"dz8_{j}", bufs=2))
            zpairs = (zhi, zlo)
            for idx, h in enumerate(ORDER):
                qk = qk_next.pop(h) if h in qk_next else proj_head(b, h)
                attention(b, h, qk, zpairs, narrow=(h // 2 == 2))
            qk_next = {}
            if b + 1 < BPC:
                # emit next batch's first projection before the output
                # projection so the PE has work while the last z normalizes
                qk_next[ORDER[0]] = proj_head(b + 1, ORDER[0])
            final_proj(b, zpairs, jlast=2)


def _get_runner(with_bias=False):
    """Build (once per variant) a jitted shard_map executing the NEFF."""
    key = ("runner", with_bias)
    if key in _CACHE:
        return _CACHE[key]

    import jax
    from jax.experimental.shard_map import shard_map
    from jax.sharding import Mesh, NamedSharding, PartitionSpec
    from concourse import mybir
    from concourse.bass2jax import (
        _bass_exec_p, install_neuronx_cc_hook, partition_id_tensor)

    nc = _build(with_bias=with_bias)
    install_neuronx_cc_hook()

    partition_name = (
        nc.partition_id_tensor.name if nc.partition_id_tensor else None)
    in_names, out_names, out_avals, zero_outs = [], [], [], []
    for alloc in nc.m.functions[0].allocations:
        if not isinstance(alloc, mybir.MemoryLocationSet):
            continue
        name = alloc.memorylocations[0].name
        if alloc.kind == "ExternalInput":
            if name != partition_name:
                in_names.append(name)
        elif alloc.kind == "ExternalOutput":
            out_names.append(name)
            shape = tuple(alloc.tensor_shape)
            dtype = mybir.dt.np(alloc.dtype)
            out_avals.append(jax.core.ShapedArray(shape, dtype))
            zero_outs.append(np.zeros(shape, dtype))
    n_params = len(in_names)
    all_in_names = in_names + out_names
    if partition_name is not None:
        all_in_names = all_in_names + [partition_name]

    def _bass_body(*args):
        operands = list(args)
        if partition_name is not None:
            operands.append(partition_id_tensor())
        outs = _bass_exec_p.bind(
            *operands,
            out_avals=tuple(out_avals),
            in_names=tuple(all_in_names),
            out_names=tuple(out_names),
            lowering_input_output_aliases=(),
            sim_require_finite=True,
            sim_require_nnan=True,
            nc=nc,
        )
        return tuple(outs)

    devices = jax.devices()[:NCORES]
    mesh = Mesh(np.asarray(devices), ("core",))
    spec = PartitionSpec("core")
    rspec = PartitionSpec()          # replicated (weights/biases)
    sharding = NamedSharding(mesh, spec)
    rsharding = NamedSharding(mesh, rspec)
    n_outs = len(out_names)
    # x8 is per-core data; everything else is identical across cores
    in_specs = tuple(spec if nm == "x8" else rspec for nm in in_names)
    jitted = jax.jit(
        shard_map(
            _bass_body, mesh=mesh,
            in_specs=in_specs + (spec,) * n_outs,
            out_specs=(spec,) * n_outs,
            check_rep=False,
        ),
        keep_unused=True,
    )
    zeros_dev = [
        jax.device_put(np.concatenate([z] * NCORES, axis=0), sharding)
        for z in zero_outs
    ]
    runner = {
        "jitted": jitted, "in_names": in_names, "out_names": out_names,
        "sharding": sharding, "rsharding": rsharding,
        "zeros_dev": zeros_dev, "jax": jax,
    }
    _CACHE[key] = runner
    return runner


def _split8(a, s):
    """fp8 residual split: return (hi, lo) e4m3 arrays with hi+lo ~= s*a."""
    import ml_dtypes
    E4 = ml_dtypes.float8_e4m3
    sa = (s * a).astype(np.float32)
    hi = sa.astype(E4)
    lo = (sa - hi.astype(np.float32)).astype(E4)
    return hi, lo


def _prep_inputs(x, Wq, bq, Wk, bk, Wv, bv, Wo, bo):
    """Host-side prep: arrays keyed by NEFF input name."""
    import ml_dtypes
    x = np.asarray(x, dtype=np.float32)
    Wq, Wk, Wv, Wo = (np.asarray(w, dtype=np.float32) for w in (Wq, Wk, Wv, Wo))
    bq, bk, bv, bo = (np.asarray(v, dtype=np.float32) for v in (bq, bk, bv, bo))

    # x: [B,N,E] -> per-core xT [E,T] -> x8[term, p, c, i, t] fp8 at scale 8
    xT = x.reshape(NCORES, T, E).transpose(0, 2, 1)     # [NC, E, T]
    xh, xl = _split8(xT, 8.0)
    # [NC, E, T] -> [NC, c, 2, 128, T] -> [NC, 128, c, 2, T]
    def xlay(a):
        return np.ascontiguousarray(
            a.reshape(NCORES, KT2, 2, 128, T).transpose(0, 3, 1, 2, 4))
    # concat over cores on the leading (sharded) axis
    x8 = np.stack([xlay(xh), xlay(xl)], axis=1).reshape(
        NCORES * 2, 128, KT2, 2, T)

    # Wq/Wk: [E, E] -> [2(qk), 128, H, 2(term), KT2, 2, D] fp8 at scale 32
    def wlay(W):
        hi, lo = _split8(W, 32.0)
        # rows k = c*256 + i*128 + p
        def lay(a):
            return a.reshape(KT2, 2, 128, H, D).transpose(2, 3, 0, 1, 4)
        # -> [128, H, KT2, 2, D]; stack terms -> [128, H, 2, KT2, 2, D]
        return np.stack([lay(hi), lay(lo)], axis=2)
    wqk = np.ascontiguousarray(np.stack([wlay(Wq), wlay(Wk)], axis=0))

    # Wv: [E, E] -> [2(term), 128, KT2, 2, E] at scale 32
    vh, vl = _split8(Wv, 32.0)
    def vlay(a):
        return a.reshape(KT2, 2, 128, E).transpose(2, 0, 1, 3)
    wv8 = np.ascontiguousarray(np.stack([vlay(vh), vlay(vl)], axis=0))

    # Wo: [E, E] -> [2(term), D, H/2(pair), 2, E] at scale 32; slot i of
    # pair j holds rows for head 2j+i
    oh, ol = _split8(Wo, 32.0)
    def olay(a):
        return a.reshape(H // 2, 2, D, E).transpose(2, 0, 1, 3)
    wo8 = np.ascontiguousarray(np.stack([olay(oh), olay(ol)], axis=0))

    bqk = np.ascontiguousarray(
        np.concatenate([bq.reshape(H, D).T, bk.reshape(H, D).T], axis=1))

    return {
        "x8": x8, "wqk8": wqk, "wv8": wv8, "wo8": wo8,
        "bqk": bqk,
        "bv1": np.ascontiguousarray(SXW * bv.reshape(1, E)),
        "bo1": np.ascontiguousarray(256.0 * 32.0 * bo.reshape(1, E)),
    }


def _run(inputs, device_resident=None, with_bias=False):
    r = _get_runner(with_bias)
    args = []
    for name in r["in_names"]:
        if device_resident is not None and name in device_resident:
            args.append(device_resident[name])
        else:
            args.append(inputs[name])
    outs = r["jitted"](*args, *r["zeros_dev"])
    return {name: outs[i] for i, name in enumerate(r["out_names"])}


def _weights_on_device(inputs, with_bias=False):
    """device_put the (replicated) weight/bias arrays once per unique value."""
    import hashlib
    r = _get_runner(with_bias)
    key = hashlib.sha1()
    for name in sorted(inputs):
        if name == "x8":
            continue
        a = inputs[name]
        key.update(name.encode())
        key.update(a.shape.__repr__().encode())
        key.update(a.tobytes())
    key = key.hexdigest()
    cached = _CACHE.get("weights_dev")
    if cached is not None and cached[0] == key:
        return cached[1]
    dev = {
        name: r["jax"].device_put(a, r["rsharding"])
        for name, a in inputs.items() if name != "x8"
    }
    _CACHE["weights_dev"] = (key, dev)
    return dev


def kernel(x, Wq, bq, Wk, bk, Wv, bv, Wo, bo):
    with_bias = any(
        np.any(np.asarray(v)) for v in (bq, bk, bv, bo))
    inputs = _prep_inputs(x, Wq, bq, Wk, bk, Wv, bv, Wo, bo)
    dev = _weights_on_device(inputs, with_bias)
    outs = _run(inputs, dev, with_bias)
    out = np.asarray(outs["out"])          # [NCORES*T, E]
    return out.reshape(B, N, E)


def bench(x, Wq, bq, Wk, bk, Wv, bv, Wo, bo, iters=20):
    """Time repeated executions with all inputs device-resident."""
    import time
    r = _get_runner()
    inputs = _prep_inputs(x, Wq, bq, Wk, bk, Wv, bv, Wo, bo)
    dev = _weights_on_device(inputs)
    dev = dict(dev)
    dev["x8"] = r["jax"].device_put(inputs["x8"], r["sharding"])

    out = _run(inputs, dev)
    list(out.values())[0].block_until_ready()

    t0 = time.time()
    last = None
    for _ in range(iters):
        last = _run(inputs, dev)
    for v in last.values():
        v.block_until_ready()
    dt = (time.time() - t0) / iters
    return dt


# revision 39
# speedup vs baseline: 1.0764x; 1.0048x over previous
#!/usr/bin/env python3
"""Multi-head attention (B=16, N=1024, E=768, H=8, softmax-then-scale variant)
as a Bass/Tile kernel on 8 TRN2 NeuronCores, data-parallel over the batch.

Per core (2 batch elements, T=2048 tokens):
  - QK projections and the V build run as fp8(e4m3) DoubleRow matmuls with a
    2-term residual split of both operands (a ~= a8 + da8 at a common scale;
    3 of the 4 cross terms are kept, the lo*lo term ~0.1% is dropped). The
    host supplies x8/dx8 at scale 8 and W8/dWlo at scale 32; the 1/256
    descale is folded into the PSUM->SBUF copies. DoubleRow contracts
    2x128 K per instruction at 0.5 cycles/row, so each term runs at 4x the
    fp32r rate and the 3-term total is 0.75x.
  - energy / attn@V / output projection stay fp32r (full-rate PE): exp
    values span e^54 so fp8 can't represent the attention weights, and the
    softmax amplifies any q/k quantization into argmax flips.
  - loop over batch b, then head h:
      energy^T per ktile: lhsT = K^T slice [96,128], rhs = Q^T [96,512]
      exp on ScalarE (no max subtraction: |energy| <~ 60 fits fp32 exp)
      attn@V flash-style: lhsT = Vhat [128, 97] (V cols for head h + a
        sqrt(E) constant column so row 96 accumulates sqrt(E)*sumexp),
        rhs = expT [128,512], accumulated over 8 k-tiles -> zT [97, 1024]
      normalize: recip = 1/zT[96] (DVE), replicated across partitions by
        the gpsimd partition_broadcast custom op, z_h = zT[0:96] * recip
    then output projection for batch b: R = sum_h z_h^T.T @ Wo_h + 1^T bo
"""
import os
import sys

sys.path.insert(0, "/opt/trn_rl_repo")

import numpy as np

B, N, E, H, D = 16, 1024, 768, 8, 96
NCORES = 8
BPC = B // NCORES          # batch elements per core
T = BPC * N                # tokens per core
KT2 = E // 256             # DoubleRow k-tiles over embedding dim (3)
MT = T // 128              # token tiles per core (16)
NKT = N // 128             # k-tiles over sequence (8)
SXW = 256.0                # x scale (8) * W scale (32)

_CACHE = {}


def _build(with_bias=True):
    import concourse.tile as tile
    from concourse import bacc, mybir

    f32 = mybir.dt.float32
    f32r = mybir.dt.float32r
    f8 = mybir.dt.float8e4

    nc = bacc.Bacc("TRN2", target_bir_lowering=False, debug=False)

    # fp8 operand pairs (hi, lo) for x, Wq/Wk, Wv; f32r elsewhere. Layouts
    # are pre-packed on the host for DoubleRow ([partition, 2, free] slices).
    x8_d = nc.dram_tensor("x8", [2, 128, KT2, 2, T], f8, kind="ExternalInput").ap()
    wqk_d = nc.dram_tensor("wqk8", [2, 128, H, 2, KT2, 2, D], f8,
                           kind="ExternalInput").ap()
    wv_d = nc.dram_tensor("wv8", [2, 128, KT2, 2, E], f8,
                          kind="ExternalInput").ap()
    wo_d = nc.dram_tensor("wo8", [2, D, H // 2, 2, E], f8,
                          kind="ExternalInput").ap()
    bqk_d = nc.dram_tensor("bqk", [D, 2 * H], f32, kind="ExternalInput").ap()
    bv_d = nc.dram_tensor("bv1", [1, E], f32r, kind="ExternalInput").ap()
    bo_d = nc.dram_tensor("bo1", [1, E], f32r, kind="ExternalInput").ap()
    out_d = nc.dram_tensor("out", [T, E], f32, kind="ExternalOutput").ap()

    with tile.TileContext(nc) as tc:
        _body(nc, tc, mybir,
              x8_d, wqk_d, wv_d, wo_d, bqk_d, bv_d, bo_d, out_d,
              with_bias)

    nc.compile()
    return nc


def _body(nc, tc, mybir,
          x8_d, wqk_d, wv_d, wo_d, bqk_d, bv_d, bo_d, out_d,
          with_bias):
    from contextlib import ExitStack
    from concourse import library_config

    f32 = mybir.dt.float32
    f32r = mybir.dt.float32r
    f8 = mybir.dt.float8e4
    Exp = mybir.ActivationFunctionType.Exp
    ADD = mybir.AluOpType.add
    MULT = mybir.AluOpType.mult
    DR = mybir.MatmulPerfMode.DoubleRow
    SUB = mybir.AluOpType.subtract
    # the Vhat constant column is sqrt(E)/SZ so the normalize reciprocal
    # yields SZ/(sqrt(E)*sumexp): z lands pre-scaled by SZ for fp8 storage
    SZ = 256.0
    SQRT_E = float(np.float32(np.sqrt(E))) / SZ
    INV = 1.0 / SXW
    INVO = 1.0 / (SZ * 32.0)   # descale for the output projection copy

    ctx = ExitStack()
    with ctx:
        persist = ctx.enter_context(tc.tile_pool(name="persist", bufs=1))
        qkpool = ctx.enter_context(tc.tile_pool(name="qkpool", bufs=1))
        projp = ctx.enter_context(tc.tile_pool(name="projp", bufs=2, space="PSUM"))
        epp = ctx.enter_context(tc.tile_pool(name="epp", bufs=2, space="PSUM"))
        zp = ctx.enter_context(tc.tile_pool(name="zp", bufs=2, space="PSUM"))

        xt = []                 # [(hi tiles), (lo tiles)] per KT2
        vhat = []
        wo8 = []
        state = {}

        # ---------------- helpers ----------------
        def proj_head(b, h):
            """Q^T/K^T for (b, h): 9 DoubleRow fp8 matmuls per 512-col chunk
            (terms W8*x8 + W8*dx8 + dWlo*x8), then a descaling copy."""
            tok0 = b * N
            qk = {}
            for nm in ("q", "k"):
                qk[nm] = qkpool.tile([D, N], f32r, name=f"{nm}t", tag=f"{nm}t",
                                     bufs=2)
            for wi, nm in enumerate(("q", "k")):
                wtile = state[f"w{nm}8"]
                qt = qk[nm]
                for tc2 in range(N // 512):
                    pq = projp.tile([128, 512], f32, name="pp", tag="pp")
                    sl = slice(tok0 + tc2 * 512, tok0 + (tc2 + 1) * 512)
                    first = True
                    for wt, xterm in ((0, 0), (0, 1), (1, 0)):
                        for c in range(KT2):
                            nc.tensor.matmul(
                                pq[0:D, :],
                                wtile[:, h][:, wt][:, c],
                                xt[xterm][c][:, :, sl],
                                start=first,
                                stop=(wt == 1 and c == KT2 - 1),
                                perf_mode=DR,
                            )
                            first = False
                    if with_bias:
                        nc.vector.tensor_scalar(
                            out=qt[:, tc2 * 512:(tc2 + 1) * 512],
                            in0=pq[0:D, :],
                            scalar1=INV,
                            scalar2=state["bqk_t"][:, wi * H + h:wi * H + h + 1],
                            op0=MULT, op1=ADD,
                        )
                    else:
                        nc.vector.tensor_scalar(
                            out=qt[:, tc2 * 512:(tc2 + 1) * 512],
                            in0=pq[0:D, :],
                            scalar1=INV, scalar2=None, op0=MULT,
                        )
            return qk

        def attention(b, h, qk, zpairs, narrow=False):
            """energy -> exp -> attn@V -> normalized fp8 z split for (b, h)."""
            zT = zp.tile([128, N], f32, name="zT", tag="zT")
            for kt in range(NKT):
                ext = expp.tile([128, N], f32r, name="ext", tag="ext")
                for qc in range(2):
                    ep = epp.tile([128, 512], f32, name="ep", tag="ep")
                    nc.tensor.matmul(
                        ep,
                        qk["k"][:, kt * 128:(kt + 1) * 128],
                        qk["q"][:, qc * 512:(qc + 1) * 512],
                        start=True, stop=True,
                    )
                    nc.scalar.activation(
                        out=ext[:, qc * 512:(qc + 1) * 512], in_=ep, func=Exp)
                    nc.tensor.matmul(
                        zT[0:D + 1, qc * 512:(qc + 1) * 512],
                        vhat[b * NKT + kt][:, h, :],
                        ext[:, qc * 512:(qc + 1) * 512],
                        start=(kt == 0), stop=(kt == NKT - 1),
                    )

            # normalize: z = SZ * zT[0:D] / (sqrt(E)*sumexp), then split into
            # fp8 hi/lo for the DoubleRow output projection. Wide
            # reciprocal+broadcast for early heads (throughput); narrow
            # per-chunk chains for the close pair (latency — the output
            # projection close matmuls wait on these).
            zhi, zlo = zpairs
            pj, slot = h // 2, h % 2
            nchunk = 2 if narrow else 1
            w = N // nchunk
            sfx = "n" if narrow else "w"
            rbs = []
            # all recips, then all broadcasts, then the splits: the second
            # chunk's chain overlaps the first chunk's split ops
            for ch in range(nchunk):
                csl = slice(ch * w, (ch + 1) * w)
                recip = rbp.tile([1, w], f32, name="recip", tag=f"recip{sfx}",
                                 bufs=2)
                nc.vector.reciprocal(out=recip, in_=zT[D:D + 1, csl])
                rbs.append(recip)
            for ch in range(nchunk):
                rb = rbp.tile([D, w], f32, name="rb", tag=f"rb{sfx}",
                              bufs=2)
                nc.gpsimd.partition_broadcast(out_ap=rb, in_ap=rbs[ch])
                rbs[ch] = rb
            for ch in range(nchunk):
                rb = rbs[ch]
                for qc in range(w // 512):
                    sl = slice(ch * w + qc * 512, ch * w + (qc + 1) * 512)
                    t = rbp.tile([D, 512], f32, name="zt_t", tag="zt_t")
                    nc.vector.tensor_mul(
                        out=t, in0=zT[0:D, sl], in1=rb[:, qc * 512:(qc + 1) * 512])
                    nc.vector.tensor_copy(out=zhi[pj][:, slot, sl], in_=t)
                    nc.vector.tensor_tensor(
                        out=zlo[pj][:, slot, sl], in0=t,
                        in1=zhi[pj][:, slot, sl], op=SUB)

        def final_proj(b, zpairs, jlast):
            """fp8 DoubleRow output projection over head pairs, software-
            pipelined: the early-ready pairs of several groups are accumulated
            before the first jlast-pair matmul so the PE has work while the
            last heads' normalize chains still run."""
            zhi, zlo = zpairs
            tok0 = b * N
            groups = [(mt, half) for mt in range(NKT) for half in range(2)]
            DEPTH = 6 if b == BPC - 1 else 5
            NP = H // 2
            jopen = [j for j in range(NP) if j != jlast]
            prs = {}
            ros = {}

            def open_group(g):
                mt, half = groups[g]
                k = g % DEPTH
                if k < 2:
                    pr = projp.tile([128, 384], f32, name="pp", tag="pp")
                elif k < 4:
                    pr = epp.tile([128, 384], f32, name="fep", tag="ep")
                else:
                    pr = zp.tile([128, 384], f32, name="fzt", tag="zT")
                cols = slice(half * 384, (half + 1) * 384)
                msl = slice(mt * 128, (mt + 1) * 128)
                first = True
                for j in jopen:
                    for zt, wt in ((zhi[j], 0), (zlo[j], 0), (zhi[j], 1)):
                        nc.tensor.matmul(
                            pr, zt[:, :, msl], wo8[wt][j][:, :, cols],
                            start=first, stop=False, perf_mode=DR,
                        )
                        first = False
                prs[g] = pr

            for g in range(min(DEPTH, len(groups))):
                open_group(g)
            for g, (mt, half) in enumerate(groups):
                pr = prs.pop(g)
                cols = slice(half * 384, (half + 1) * 384)
                msl = slice(mt * 128, (mt + 1) * 128)
                j = jlast
                for ti, (zt, wt) in enumerate(
                        ((zhi[j], 0), (zlo[j], 0), (zhi[j], 1))):
                    nc.tensor.matmul(
                        pr, zt[:, :, msl], wo8[wt][j][:, :, cols],
                        start=False,
                        stop=(ti == 2 and not with_bias), perf_mode=DR,
                    )
                if with_bias:
                    # bo is pre-scaled by SZ*32 on the host
                    nc.tensor.matmul(
                        pr, onescol_r, state["bor"][:, cols],
                        start=False, stop=True,
                    )
                if half == 0:
                    ros[mt] = rop.tile([128, E], f32, name="ro", tag="ro")
                if g % 2 == 0:
                    nc.scalar.mul(out=ros[mt][:, cols], in_=pr, mul=INVO)
                else:
                    nc.vector.tensor_scalar(
                        out=ros[mt][:, cols], in0=pr,
                        scalar1=INVO, scalar2=None, op0=MULT)
                if g + DEPTH < len(groups):
                    open_group(g + DEPTH)
                # alternate the output stores across the two HWDGE queues so
                # the end-of-kernel DMA tail is not serialized on one queue
                dma_eng = nc.sync if g % 2 == 0 else nc.scalar
                dma_eng.dma_start(
                    out=out_d[tok0 + mt * 128:tok0 + (mt + 1) * 128, cols],
                    in_=ros[mt][:, cols])
                if half == 1:
                    ros.pop(mt)

        # ---------------- phase 0: loads + Vhat ----------------
        qk00 = None
        with tc.tile_pool(name="wvpool", bufs=1) as wvpool:
            for term in range(2):
                tiles = []
                for c in range(KT2):
                    tiles.append(persist.tile([128, 2, T], f8,
                                              name=f"xt{term}_{c}",
                                              tag=f"xt{term}_{c}"))
                xt.append(tiles)

            def load_x_quarter(q):
                for hf in range(2):
                    sl = slice(q * 512 + hf * 256, q * 512 + (hf + 1) * 256)
                    for term in range(2):
                        for c in range(KT2):
                            nc.sync.dma_start(
                                out=xt[term][c][:, :, sl],
                                in_=x8_d[term][:, c][:, :, sl])

            # constants
            ones_f = persist.tile([1, 128], f32, name="ones_f", tag="ones_f")
            nc.vector.memset(ones_f, 1.0)
            onescol_r = persist.tile([1, 128], f32r, name="ones_r", tag="ones_r")
            nc.vector.tensor_copy(out=onescol_r, in_=ones_f)
            c27f = persist.tile([128, 1], f32, name="c27f", tag="c27f")
            nc.vector.memset(c27f, SQRT_E)
            c27r = persist.tile([128, 1], f32r, name="c27r", tag="c27r")
            nc.vector.tensor_copy(out=c27r, in_=c27f)

            # first x quarter interleaved with Wv so the Vhat(0) psum
            # group can start accumulating early; hi terms are loaded before
            # lo terms to match the matmul emission order within each group
            wv = [[], []]
            for term in range(2):
                for c in range(KT2):
                    nc.sync.dma_start(
                        out=xt[term][c][:, :, 0:256],
                        in_=x8_d[term][:, c][:, :, 0:256])
                    wvc = wvpool.tile([128, 2, E], f8, name=f"wv{term}_{c}",
                                      tag=f"wv{term}_{c}")
                    nc.gpsimd.dma_start(out=wvc, in_=wv_d[term][:, c])
                    wv[term].append(wvc)
            for term in range(2):
                for c in range(KT2):
                    nc.sync.dma_start(
                        out=xt[term][c][:, :, 256:512],
                        in_=x8_d[term][:, c][:, :, 256:512])

            # gpsimd ucode library with partition_broadcast (needed by the
            # first normalize; emitted after the Wv loads so it does not
            # head-of-line block the gpsimd DMA queue at startup)
            nc.gpsimd.load_library(library_config.attn)

            # persistent Wq/Wk fp8 tiles (one DMA per head per q/k)
            for nm, wi in (("q", 0), ("k", 1)):
                wt8 = persist.tile([128, H, 2, KT2, 2, D], f8,
                                   name=f"w{nm}8", tag=f"w{nm}8")
                for h in (6, 7, 0, 1, 2, 3, 4, 5):
                    nc.gpsimd.dma_start(out=wt8[:, h], in_=wqk_d[wi][:, h])
                state[f"w{nm}8"] = wt8

            # biases
            bqk_t = persist.tile([D, 2 * H], f32, name="bqk_t", tag="bqk_t")
            nc.gpsimd.dma_start(out=bqk_t, in_=bqk_d)
            state["bqk_t"] = bqk_t
            bvr = persist.tile([1, E], f32r, name="bvr", tag="bvr")
            nc.gpsimd.dma_start(out=bvr, in_=bv_d)

            def build_vhat(mt):
                # Vhat[mt] : [128 tokens, H, D+1]; column D holds sqrt(E)
                vh = persist.tile([128, H, D + 1], f32r, name=f"vhat{mt}",
                                  tag=f"vhat{mt}")
                msl = slice(mt * 128, (mt + 1) * 128)
                for half in range(2):  # heads 0-3 / 4-7 (384 cols each)
                    pv = projp.tile([128, 512], f32, name="pp", tag="pp")
                    cols = slice(half * 4 * D, (half + 1) * 4 * D)
                    terms = ((0, 0), (1, 0), (0, 1))
                    for ti, (wt, xterm) in enumerate(terms):
                        for c in range(KT2):
                            nc.tensor.matmul(
                                pv[:, 0:4 * D],
                                xt[xterm][c][:, :, msl],
                                wv[wt][c][:, :, cols],
                                start=(ti == 0 and c == 0),
                                stop=(with_bias is False and ti == 2
                                      and c == KT2 - 1),
                                perf_mode=DR,
                            )
                    if with_bias:
                        # bv is pre-scaled by SXW on the host
                        nc.tensor.matmul(
                            pv[:, 0:4 * D], onescol_r, bvr[:, cols],
                            start=False, stop=True,
                        )
                    if mt < 8:
                        nc.scalar.mul(
                            out=vh[:, half * 4:(half + 1) * 4, 0:D],
                            in_=pv[:, 0:4 * D].rearrange("p (h d) -> p h d",
                                                         h=4),
                            mul=INV,
                        )
                    else:
                        # later Vhat copies on DVE: on ACT they would queue
                        # ahead of the first attention exps
                        nc.vector.tensor_scalar(
                            out=vh[:, half * 4:(half + 1) * 4, 0:D],
                            in0=pv[:, 0:4 * D].rearrange("p (h d) -> p h d",
                                                         h=4),
                            scalar1=INV, scalar2=None, op0=MULT,
                        )
                nc.vector.tensor_copy(
                    out=vh[:, :, D:D + 1],
                    in_=c27r.to_broadcast([128, H, 1]),
                )
                vhat.append(vh)

            # interleave: quarters 0-1 -> Vhat 0-7, then the first head
            # projection (keeps the PE busy while quarters 2-3 stream in)
            for q in range(2):
                if q > 0:
                    load_x_quarter(q)
                for mt in range(4 * q, 4 * q + 4):
                    build_vhat(mt)
            qk00 = proj_head(0, 6)
            for q in range(2, 4):
                load_x_quarter(q)
                for mt in range(4 * q, 4 * q + 4):
                    build_vhat(mt)

        # stage + wv pools released; later pools reuse their space
        expp = ctx.enter_context(tc.tile_pool(name="expp", bufs=3))
        rbp = ctx.enter_context(tc.tile_pool(name="rbp", bufs=2))
        rop = ctx.enter_context(tc.tile_pool(name="rop", bufs=2))
        ztpool = ctx.enter_context(tc.tile_pool(name="ztpool", bufs=1))
        wopool = ctx.enter_context(tc.tile_pool(name="wopool", bufs=1))

        # Wo -> fp8 hi/lo per-head-pair tiles + bo (phase 2 operands)
        for term in range(2):
            tiles = []
            for j in range(H // 2):
                woj = wopool.tile([D, 2, E], f8, name=f"wo{term}_{j}",
                                  tag=f"wo{term}_{j}")
                nc.gpsimd.dma_start(out=woj, in_=wo_d[term][:, j])
                tiles.append(woj)
            wo8.append(tiles)
        if with_bias:
            bor = wopool.tile([1, E], f32r, name="bor", tag="bor")
            nc.gpsimd.dma_start(out=bor, in_=bo_d)
            state["bor"] = bor

        # ---------------- phases 1+2, batch-major, software-pipelined ------
        # head order: pair 3 (h6,h7) first so it is ready long before the
        # output projection; pair 2 (h4,h5) finishes last and is the close
        # pair there. Each head's projection is emitted one step ahead so
        # the energy matmuls never wait on the PSUM->SBUF copy latency.
        ORDER = (6, 7, 0, 1, 2, 3, 4, 5)
        qk_next = {ORDER[0]: qk00}
        for b in range(BPC):
            zhi, zlo = [], []
            for j in range(H // 2):
                zhi.append(ztpool.tile([D, 2, N], f8, name=f"z8_{j}",
                                       tag=f"z8_{j}", bufs=2))
                zlo.append(ztpool.tile([D, 2, N], f8, name=f"dz8_{j}",
                                       tag=f"dz8_{j}", bufs=2))
            zpairs = (zhi, zlo)
            for idx, h in enumerate(ORDER):
                qk = qk_next.pop(h) if h in qk_next else proj_head(b, h)
                attention(b, h, qk, zpairs, narrow=(h // 2 == 2))
            qk_next = {}
            if b + 1 < BPC:
                # emit next batch's first projection before the output
                # projection so the PE has work while the last z normalizes
                qk_next[ORDER[0]] = proj_head(b + 1, ORDER[0])
            final_proj(b, zpairs, jlast=2)


def _get_runner(with_bias=False):
    """Build (once per variant) a jitted shard_map executing the NEFF."""
    key = ("runner", with_bias)
    if key in _CACHE:
        return _CACHE[key]

    import jax
    from jax.experimental.shard_map import shard_map
    from jax.sharding import Mesh, NamedSharding, PartitionSpec
    from concourse import mybir
    from concourse.bass2jax import (
        _bass_exec_p, install_neuronx_cc_hook, partition_id_tensor)

    nc = _build(with_bias=with_bias)
    install_neuronx_cc_hook()

    partition_name = (
        nc.partition_id_tensor.name if nc.partition_id_tensor else None)
    in_names, out_names, out_avals, zero_outs = [], [], [], []
    for alloc in nc.m.functions[0].allocations:
        if not isinstance(alloc, mybir.MemoryLocationSet):
            continue
        name = alloc.memorylocations[0].name
        if alloc.kind == "ExternalInput":
            if name != partition_name:
                in_names.append(name)
        elif alloc.kind == "ExternalOutput":
            out_names.append(name)
            shape = tuple(alloc.tensor_shape)
            dtype = mybir.dt.np(alloc.dtype)
            out_avals.append(jax.core.ShapedArray(shape, dtype))
            zero_outs.append(np.zeros(shape, dtype))
    n_params = len(in_names)
    all_in_names = in_names + out_names
    if partition_name is not None:
        all_in_names = all_in_names + [partition_name]

    def _bass_body(*args):
        operands = list(args)
        if partition_name is not None:
            operands.append(partition_id_tensor())
        outs = _bass_exec_p.bind(
            *operands,
            out_avals=tuple(out_avals),
            in_names=tuple(all_in_names),
            out_names=tuple(out_names),
            lowering_input_output_aliases=(),
            sim_require_finite=True,
            sim_require_nnan=True,
            nc=nc,
        )
        return tuple(outs)

    devices = jax.devices()[:NCORES]
    mesh = Mesh(np.asarray(devices), ("core",))
    spec = PartitionSpec("core")
    rspec = PartitionSpec()          # replicated (weights/biases)
    sharding = NamedSharding(mesh, spec)
    rsharding = NamedSharding(mesh, rspec)
    n_outs = len(out_names)
    # x8 is per-core data; everything else is identical across cores
    in_specs = tuple(spec if nm == "x8" else rspec for nm in in_names)
    jitted = jax.jit(
        shard_map(
            _bass_body, mesh=mesh,
            in_specs=in_specs + (spec,) * n_outs,
            out_specs=(spec,) * n_outs,
            check_rep=False,
        ),
        keep_unused=True,
    )
    zeros_dev = [
        jax.device_put(np.concatenate([z] * NCORES, axis=0), sharding)
        for z in zero_outs
    ]
    runner = {
        "jitted": jitted, "in_names": in_names, "out_names": out_names,
        "sharding": sharding, "rsharding": rsharding,
        "zeros_dev": zeros_dev, "jax": jax,
    }
    _CACHE[key] = runner
    return runner


def _split8(a, s):
    """fp8 residual split: return (hi, lo) e4m3 arrays with hi+lo ~= s*a."""
    import ml_dtypes
    E4 = ml_dtypes.float8_e4m3
    sa = (s * a).astype(np.float32)
    hi = sa.astype(E4)
    lo = (sa - hi.astype(np.float32)).astype(E4)
    return hi, lo


def _prep_inputs(x, Wq, bq, Wk, bk, Wv, bv, Wo, bo):
    """Host-side prep: arrays keyed by NEFF input name."""
    import ml_dtypes
    x = np.asarray(x, dtype=np.float32)
    Wq, Wk, Wv, Wo = (np.asarray(w, dtype=np.float32) for w in (Wq, Wk, Wv, Wo))
    bq, bk, bv, bo = (np.asarray(v, dtype=np.float32) for v in (bq, bk, bv, bo))

    # x: [B,N,E] -> per-core xT [E,T] -> x8[term, p, c, i, t] fp8 at scale 8
    xT = x.reshape(NCORES, T, E).transpose(0, 2, 1)     # [NC, E, T]
    xh, xl = _split8(xT, 8.0)
    # [NC, E, T] -> [NC, c, 2, 128, T] -> [NC, 128, c, 2, T]
    def xlay(a):
        return np.ascontiguousarray(
            a.reshape(NCORES, KT2, 2, 128, T).transpose(0, 3, 1, 2, 4))
    # concat over cores on the leading (sharded) axis
    x8 = np.stack([xlay(xh), xlay(xl)], axis=1).reshape(
        NCORES * 2, 128, KT2, 2, T)

    # Wq/Wk: [E, E] -> [2(qk), 128, H, 2(term), KT2, 2, D] fp8 at scale 32
    def wlay(W):
        hi, lo = _split8(W, 32.0)
        # rows k = c*256 + i*128 + p
        def lay(a):
            return a.reshape(KT2, 2, 128, H, D).transpose(2, 3, 0, 1, 4)
        # -> [128, H, KT2, 2, D]; stack terms -> [128, H, 2, KT2, 2, D]
        return np.stack([lay(hi), lay(lo)], axis=2)
    wqk = np.ascontiguousarray(np.stack([wlay(Wq), wlay(Wk)], axis=0))

    # Wv: [E, E] -> [2(term), 128, KT2, 2, E] at scale 32
    vh, vl = _split8(Wv, 32.0)
    def vlay(a):
        return a.reshape(KT2, 2, 128, E).transpose(2, 0, 1, 3)
    wv8 = np.ascontiguousarray(np.stack([vlay(vh), vlay(vl)], axis=0))

    # Wo: [E, E] -> [2(term), D, H/2(pair), 2, E] at scale 32; slot i of
    # pair j holds rows for head 2j+i
    oh, ol = _split8(Wo, 32.0)
    def olay(a):
        return a.reshape(H // 2, 2, D, E).transpose(2, 0, 1, 3)
    wo8 = np.ascontiguousarray(np.stack([olay(oh), olay(ol)], axis=0))

    bqk = np.ascontiguousarray(
        np.concatenate([bq.reshape(H, D).T, bk.reshape(H, D).T], axis=1))

    return {
        "x8": x8, "wqk8": wqk, "wv8": wv8, "wo8": wo8,
        "bqk": bqk,
        "bv1": np.ascontiguousarray(SXW * bv.reshape(1, E)),
        "bo1": np.ascontiguousarray(256.0 * 32.0 * bo.reshape(1, E)),
    }


def _run(inputs, device_resident=None, with_bias=False):
    r = _get_runner(with_bias)
    args = []
    for name in r["in_names"]:
        if device_resident is not None and name in device_resident:
            args.append(device_resident[name])
        else:
            args.append(inputs[name])
    outs = r["jitted"](*args, *r["zeros_dev"])
    return {name: outs[i] for i, name in enumerate(r["out_names"])}


def _weights_on_device(inputs, with_bias=False):
    """device_put the (replicated) weight/bias arrays once per unique value."""
    import hashlib
    r = _get_runner(with_bias)
    key = hashlib.sha1()
    for name in sorted(inputs):
        if name == "x8":
            continue
        a = inputs[name]
        key.update(name.encode())
        key.update(a.shape.__repr__().encode())
        key.update(a.tobytes())
    key = key.hexdigest()
    cached = _CACHE.get("weights_dev")
    if cached is not None and cached[0] == key:
        return cached[1]
    dev = {
        name: r["jax"].device_put(a, r["rsharding"])
        for name, a in inputs.items() if name != "x8"
    }
    _CACHE["weights_dev"] = (key, dev)
    return dev


def kernel(x, Wq, bq, Wk, bk, Wv, bv, Wo, bo):
    with_bias = any(
        np.any(np.asarray(v)) for v in (bq, bk, bv, bo))
    inputs = _prep_inputs(x, Wq, bq, Wk, bk, Wv, bv, Wo, bo)
    dev = _weights_on_device(inputs, with_bias)
    outs = _run(inputs, dev, with_bias)
    out = np.asarray(outs["out"])          # [NCORES*T, E]
    return out.reshape(B, N, E)


def bench(x, Wq, bq, Wk, bk, Wv, bv, Wo, bo, iters=20):
    """Time repeated executions with all inputs device-resident."""
    import time
    r = _get_runner()
    inputs = _prep_inputs(x, Wq, bq, Wk, bk, Wv, bv, Wo, bo)
    dev = _weights_on_device(inputs)
    dev = dict(dev)
    dev["x8"] = r["jax"].device_put(inputs["x8"], r["sharding"])

    out = _run(inputs, dev)
    list(out.values())[0].block_until_ready()

    t0 = time.time()
    last = None
    for _ in range(iters):
        last = _run(inputs, dev)
    for v in last.values():
        v.block_until_ready()
    dt = (time.time() - t0) / iters
    return dt
